# revision 1
# baseline (speedup 1.0000x reference)
"""Trainium2 Bass kernel for the CAM (channel attention) module.

Computes, per batch element b:
    q = x[b].reshape(C, N)                      # C=512, N=4096
    E = q @ q.T                                 # C x C  (symmetric)
    att = softmax(rowmax(E) - E, axis=-1)       # == softmax(-E) row-wise
    out = gamma * (att @ q) + x[b]

Sharding: data-parallel over batch. 16 batch elements -> 2 per NeuronCore
across 8 cores. gamma replicated. No collectives.

Per-core kernel strategy (per batch element):
  1. DMA q into SBUF in natural layout qnat[c_part, n_free] (fp32, exact bits
     are reused for the +x residual, so this tile is never rounded).
  2. Build qT[n_part, c_free] with 128 PE [128x128] transposes; 4 transposes
     share one PSUM bank so a single [128,512] DVE copy drains them (4x fewer
     DVE ops). qT is stored as float32r: the DVE copy rounds, satisfying the
     fp32r-producer rule, and the energy matmul then runs at full PE rate
     (1 cycle/row) instead of fp32's 1/4 rate.
  3. E tiles [128, 512] accumulate in PSUM via fp32r matmuls.
  4. Column-oriented softmax avoids transposing the attention matrix:
     att_T[d, c] = exp(min_c - E[d, c]) / R_c with R_c = sum_d exp(...).
     E is symmetric so min_c (row mins) equals the column-min vector; the
     stored E tile read with d on partitions is already att_T-oriented.
     exp argument <= 0 always, so no overflow; R is clamped before the
     reciprocal so no NaN is possible.
  5. U = exp(min_c - E) in bf16 is the stationary operand of the value
     matmul against a bf16 copy of q (cast on the idle scalar engine).
     gamma/R_c (per output partition) and the +x residual are fused into one
     DVE scalar_tensor_tensor per output chunk; x enters only here, in exact
     fp32, so for gamma == 0 the kernel output is bit-exact x.
"""

import sys

import numpy as np

_REPO = "/opt/trn_rl_repo"
if _REPO not in sys.path:
    sys.path.insert(0, _REPO)

B_TOTAL, C, H, W = 16, 512, 64, 64
N = H * W          # 4096
NCORES = 8
B = B_TOTAL // NCORES  # batches per core = 2
CT = C // 128      # 4 c-tiles
NT = N // 128      # 32 n-tiles
NCH = N // 512     # 8 output column chunks

_cache = {}


def _build_program():
    import concourse.bass as bass
    import concourse.bacc as bacc
    import concourse.mybir as mybir
    import concourse.tile as tile
    from contextlib import ExitStack

    f32 = mybir.dt.float32
    f32r = mybir.dt.float32r
    bf16 = mybir.dt.bfloat16
    AX = mybir.AxisListType
    OP = mybir.AluOpType
    ACT = mybir.ActivationFunctionType

    nc = bacc.Bacc("TRN2", target_bir_lowering=False, debug=False)

    x = nc.dram_tensor("x", [B, C, N], f32, kind="ExternalInput").ap()
    g128 = nc.dram_tensor("gamma128", [128, 1], f32, kind="ExternalInput").ap()
    ident_d = nc.dram_tensor("ident", [128, 128], f32, kind="ExternalInput").ap()
    y = nc.dram_tensor("y", [B, C, N], f32, kind="ExternalOutput").ap()

    with ExitStack() as ctx:
        tc = ctx.enter_context(tile.TileContext(nc))
        const_p = ctx.enter_context(tc.tile_pool(name="const", bufs=1))
        # qnat (fp32 q) and qT (f32r transposed q) alternate through 2 slots;
        # batch b+1's qnat lands in the slot freed by batch b's qT so its DMA
        # overlaps batch b's value-matmul phase.
        big_p = ctx.enter_context(tc.tile_pool(name="big", bufs=2))
        q_p = ctx.enter_context(tc.tile_pool(name="qq", bufs=1))
        qbf_p = ctx.enter_context(tc.tile_pool(name="qbf", bufs=1))
        tmp_p = ctx.enter_context(tc.tile_pool(name="tmp", bufs=2))
        sm_p = ctx.enter_context(tc.tile_pool(name="sm", bufs=2))
        rep_p = ctx.enter_context(tc.tile_pool(name="rep", bufs=1))
        osb_p = ctx.enter_context(tc.tile_pool(name="osb", bufs=8))
        ps = ctx.enter_context(tc.tile_pool(name="ps", bufs=8, space="PSUM"))

        ident = const_p.tile([128, 128], f32, tag="ident")
        nc.sync.dma_start(ident[:], ident_d)
        gam = const_p.tile([128, 1], f32, tag="gam")
        nc.sync.dma_start(gam[:], g128)
        ones128 = const_p.tile([128, 1], bf16, tag="ones128")
        nc.gpsimd.memset(ones128[:], 1.0)
        ones1 = const_p.tile([1, 128], f32, tag="ones1")
        nc.gpsimd.memset(ones1[:], 1.0)

        # warm the PE clock during the initial DMA wait: dummy transposes of
        # the identity keep the ramp/HAM window busy so the first real
        # transposes run at full clock
        warm = ps.tile([128, 512], f32, tag="ps", name="warm")
        for w in range(8):
            nc.tensor.matmul(
                warm[:, 128 * (w % 4):128 * (w % 4 + 1)],
                ident[:],
                ident[:],
                is_transpose=True,
                skip_group_check=True,
            )

        for b in range(B):
            # ---- load q in natural layout, chunked so transposes can
            #      start as soon as the first columns land
            qnat = big_p.tile([128, CT, N], f32, tag="big")
            for t in range(CT):
                for lo, hi in [(0, 128), (128, 512)]:
                    nc.sync.dma_start(
                        qnat[:, t, lo:hi],
                        x[b, 128 * t:128 * (t + 1), lo:hi],
                    )
                for h in range(1, 8):
                    nc.sync.dma_start(
                        qnat[:, t, 512 * h:512 * (h + 1)],
                        x[b, 128 * t:128 * (t + 1), 512 * h:512 * (h + 1)],
                    )

            # ---- build qT[n_part, c_free]; 4 transposes per PSUM bank, one
            #      [128,512] DVE copy per bank (rounds to f32r)
            qt = big_p.tile([128, NT, C], f32r, tag="big")
            for t in range(CT):
                for jq in range(NT // 4):
                    tp4 = ps.tile([128, 512], f32, tag="ps")
                    for i in range(4):
                        j = 4 * jq + i
                        nc.tensor.matmul(
                            tp4[:, 128 * i:128 * (i + 1)],
                            qnat[:, t, 128 * j:128 * (j + 1)],
                            ident[:],
                            is_transpose=True,
                            skip_group_check=True,
                        )
                    nc.vector.tensor_copy(
                        qt[:, 4 * jq:4 * (jq + 1), 128 * t:128 * (t + 1)],
                        tp4[:].rearrange("p (a c) -> p a c", a=4),
                    )

            # ---- bf16 copy of q for the value matmul, on the idle scalar
            #      engine (ACT)
            qbf = qbf_p.tile([128, CT, N], bf16, tag="qbf")
            for t in range(CT):
                nc.scalar.copy(qbf[:, t, :], qnat[:, t, :])

            # ---- energy: E is symmetric, so compute only columns
            #      [lo_t:512] per row-tile (lo capped at 256: narrower f32r
            #      moving operands drop to 1/4 rate) and mirror the missing
            #      [128,128] blocks by transposing the stored ones.
            elo = [0, 128, 256, 256]
            mirrors = {0: [(0, 1), (0, 2), (0, 3)], 1: [(1, 2), (1, 3)]}
            rmins = sm_p.tile([128, CT], f32, tag="rmins")
            colrep_ps = ps.tile([128, C], f32, tag="ps")
            E = [ps.tile([128, C], f32, tag="ps", name=f"Et{t_}")
                 for t_ in range(CT)]
            for t in range(CT):
                Et = E[t]
                for j in range(NT):
                    nc.tensor.matmul(
                        Et[:, elo[t]:C],
                        qt[:, j, 128 * t:128 * (t + 1)],
                        qt[:, j, elo[t]:C],
                        start=(j == 0),
                        stop=(j == NT - 1),
                    )
                # mirror blocks sourced from tile t into later tiles' banks
                # (target regions are disjoint from their MM-written ranges,
                # so this can precede those tiles' accumulation)
                for s, tt in mirrors.get(t, []):
                    blk = sm_p.tile([128, 128], f32, tag="mirror")
                    nc.vector.tensor_copy(
                        blk[:], E[s][:, 128 * tt:128 * (tt + 1)]
                    )
                    nc.tensor.matmul(
                        E[tt][:, 128 * s:128 * (s + 1)],
                        blk[:],
                        ident[:],
                        is_transpose=True,
                        skip_group_check=True,
                    )
                # tile t of E is now complete (its own MMs + any mirrors
                # emitted in earlier iterations): fold its stats immediately
                # so only tile 3's chain trails the energy phase
                nc.vector.tensor_reduce(
                    rmins[:, t:t + 1], E[t][:], axis=AX.X, op=OP.min
                )
                tpm = ps.tile([1, 128], f32, tag="ps")
                nc.tensor.transpose(tpm[:], rmins[:, t:t + 1], ident[:])
                stT = sm_p.tile([1, 128], f32, tag="stT")
                nc.vector.tensor_copy(stT[:], tpm[:])
                nc.tensor.matmul(
                    colrep_ps[:, 128 * t:128 * (t + 1)],
                    ones1[:],
                    stT[:],
                    start=True,
                    stop=True,
                )
            colrep = rep_p.tile([128, C], f32, tag="colrep")
            nc.vector.tensor_copy(colrep[:], colrep_ps[:])

            # ---- U[d, c] = exp(min_c - E[d, c])  (<= 1, no overflow)
            U = q_p.tile([128, CT, C], bf16, tag="qq")
            for t in range(CT):
                tmp = tmp_p.tile([128, C], f32, tag="tmp")
                nc.vector.tensor_tensor(
                    tmp[:], colrep[:], E[t][:], op=OP.subtract
                )
                nc.scalar.activation(U[:, t, :], tmp[:], ACT.Exp)

            # ---- out[c, n] = scale_c * sum_d U[d, c] q[d, n] + x[c, n]
            #      R_c = sum_d U[d, c] (PE ones-reduction) is interleaved
            #      per m so the first value matmuls start sooner;
            #      scale_m = gamma / max(R, tiny) per output partition
            for m in range(CT):
                Rp = ps.tile([128, 1], f32, tag="ps")
                for k in range(CT):
                    nc.tensor.matmul(
                        Rp[:],
                        U[:, k, 128 * m:128 * (m + 1)],
                        ones128[:],
                        start=(k == 0),
                        stop=(k == CT - 1),
                    )
                Rsb = sm_p.tile([128, 1], f32, tag="rsb")
                nc.vector.tensor_scalar_max(Rsb[:], Rp[:], 1e-38)
                rec = sm_p.tile([128, 1], f32, tag="rec")
                nc.vector.reciprocal(rec[:], Rsb[:])
                sc = sm_p.tile([128, 1], f32, tag=f"scale{m}")
                nc.vector.tensor_scalar_mul(sc[:], rec[:], gam[:, 0:1])
                O = []
                for n in range(NCH):
                    On = ps.tile([128, 512], f32, tag="ps")
                    O.append(On)
                for k in range(CT):
                    for n in range(NCH):
                        nc.tensor.matmul(
                            O[n][:],
                            U[:, k, 128 * m:128 * (m + 1)],
                            qbf[:, k, 512 * n:512 * (n + 1)],
                            start=(k == 0),
                            stop=(k == CT - 1),
                            skip_group_check=True,
                        )
                for n in range(NCH):
                    osb = osb_p.tile([128, 512], f32, tag="osb")
                    nc.vector.scalar_tensor_tensor(
                        osb[:],
                        O[n][:],
                        sc[:],
                        qnat[:, m, 512 * n:512 * (n + 1)],
                        op0=OP.mult,
                        op1=OP.add,
                    )
                    nc.sync.dma_start(
                        y[b, 128 * m:128 * (m + 1), 512 * n:512 * (n + 1)],
                        osb[:],
                    )

    nc.compile()
    return nc


def get_program():
    if "nc" not in _cache:
        _cache["nc"] = _build_program()
    return _cache["nc"]


def kernel(x, gamma):
    from concourse.bass_utils import run_bass_kernel_spmd

    nc = get_program()
    xr = np.ascontiguousarray(
        np.asarray(x, dtype=np.float32).reshape(B_TOTAL, C, N)
    )
    g = np.asarray(gamma, dtype=np.float32).reshape(1)
    g128 = np.ascontiguousarray(
        np.broadcast_to(g.reshape(1, 1), (128, 1))
    ).astype(np.float32)
    ident = np.eye(128, dtype=np.float32)
    in_maps = [
        {
            "x": xr[i * B:(i + 1) * B],
            "gamma128": g128,
            "ident": ident,
        }
        for i in range(NCORES)
    ]
    res = run_bass_kernel_spmd(nc, in_maps, list(range(NCORES))).results
    y = np.concatenate([res[i]["y"] for i in range(NCORES)], axis=0)
    return y.reshape(B_TOTAL, C, H, W).astype(np.float32)



# revision 32
# speedup vs baseline: 1.5951x; 1.5951x over previous
"""Trainium2 Bass kernel for the CAM (channel attention) module.

Computes, per batch element b:
    q = x[b].reshape(C, N)                      # C=512, N=4096
    E = q @ q.T                                 # C x C  (symmetric)
    att = softmax(rowmax(E) - E, axis=-1)       # == softmax(-E) row-wise
    out = gamma * (att @ q) + x[b]

Sharding: data-parallel over batch. 16 batch elements -> 2 per NeuronCore
across 8 cores. gamma replicated. No collectives.

Per-core roofline: the shared DMA engine pool moves x in (16.8 MB fp32)
and y out (8.4 MB fp16) at ~360 GB/s -> ~70 us. Everything else is
scheduled to hide under that:

  1. x streams in as [128,2048] fp32 chunks, interleaved 2:1 between the
     two batch elements so batch 0's pipeline finishes ~17us before the
     last load. Each chunk is cast to q8 (fp8e4, feeds all matmuls) and
     x16 (fp16 residual copy) from a small rotating fp32 buffer.
  2. qT[n_part, c] built with PE fp8 transposes (1 cycle/row); 8
     transposes pack one PSUM bank in 2-byte lanes (hw: fp8 transpose
     output element step must be 2), drained by one strided copy.
  3. Energy E tiles accumulate via fp8 DoubleRow matmuls (K=256 per
     instruction, 0.5 cycles/row) in two j-halves, the first starting
     before the last drains land. Each finished E tile is min-reduced
     and copied to SBUF immediately, freeing its PSUM bank for the next
     batch's transposes.
  4. Column-softmax without transposing attention: E is symmetric so row
     mins equal column mins; U[d,c] = exp(min_c - E[d,c]) <= 1 in fp8.
     The subtract runs in place on the SBUF copy of E.
  5. Value matmul att @ q via DoubleRow fp8 into single-bank PSUM
     chunks rotating 4+ deep (batch 1's chunks also reuse the retired
     transpose/energy bank slots); R_c = sum_d U[d,c] via a PE
     ones-reduction and one fused gamma/R divide (R >= 1 structurally).
     The scale + x residual drain each chunk directly
     (scalar_tensor_tensor on DVE/Pool) or as an ACT scale-copy plus a
     2x-mode fp16 DVE add, so all three elementwise engines share the
     output stream. The lazy softmax runs per output tile: a 32x32
     block-transpose of the min matrix, four gpsimd partition
     broadcasts, one fused fp16 subtract, one exp.
  6. All loads are issued on the SP queue before any store so waiting
     stores never starve the DMA pipe.

Output is written fp16 (max rel err ~5e-4 vs the fp32 reference, gate
is 2e-2); host casts back to fp32.
"""

import sys

import numpy as np

_REPO = "/opt/trn_rl_repo"
if _REPO not in sys.path:
    sys.path.insert(0, _REPO)

B_TOTAL, C, H, W = 16, 512, 64, 64
N = H * W          # 4096
NCORES = 8
B = B_TOTAL // NCORES  # batches per core = 2
CT = C // 128      # 4 c-tiles
NT = N // 128      # 32 n-tiles

# chunk arrival order: (b, t, h); h=0 halves first per batch so the
# first energy half can start early, 2:1 interleave favoring batch 0
CHUNKS = [
    (0, 0, 0), (0, 1, 0), (1, 0, 0), (0, 2, 0), (0, 3, 0), (1, 1, 0),
    (0, 0, 1), (0, 1, 1), (1, 2, 0), (0, 2, 1), (0, 3, 1), (1, 3, 0),
    (1, 0, 1), (1, 1, 1), (1, 2, 1), (1, 3, 1),
]

_cache = {}


def _build_program():
    import concourse.bass as bass
    import concourse.bacc as bacc
    import concourse.mybir as mybir
    import concourse.tile as tile
    from contextlib import ExitStack

    f32 = mybir.dt.float32
    f16 = mybir.dt.float16
    f8 = mybir.dt.float8e4
    AX = mybir.AxisListType
    OP = mybir.AluOpType
    ACT = mybir.ActivationFunctionType
    DR = mybir.MatmulPerfMode.DoubleRow

    nc = bacc.Bacc("TRN2", target_bir_lowering=False, debug=False)

    x = nc.dram_tensor("x", [B, C, N], f32, kind="ExternalInput").ap()
    g128 = nc.dram_tensor("gamma128", [128, 1], f32, kind="ExternalInput").ap()
    ident_d = nc.dram_tensor("ident", [128, 128], f32, kind="ExternalInput").ap()
    y = nc.dram_tensor("y", [B, C, N], f16, kind="ExternalOutput").ap()

    with ExitStack() as ctx:
        tc = ctx.enter_context(tile.TileContext(nc))
        const_p = ctx.enter_context(tc.tile_pool(name="const", bufs=1))
        qraw_p = ctx.enter_context(tc.tile_pool(name="qraw", bufs=3))
        x16_p = ctx.enter_context(tc.tile_pool(name="x16", bufs=2))
        q8_p = ctx.enter_context(tc.tile_pool(name="q8", bufs=2))
        qt_p = ctx.enter_context(tc.tile_pool(name="qt", bufs=2))
        u_p = ctx.enter_context(tc.tile_pool(name="u", bufs=2))
        rep_p = ctx.enter_context(tc.tile_pool(name="rep", bufs=1))
        ecp_p = ctx.enter_context(tc.tile_pool(name="ecp", bufs=4))
        os16_p = ctx.enter_context(tc.tile_pool(name="os16", bufs=2))
        osb_p = ctx.enter_context(tc.tile_pool(name="osb", bufs=3))
        sm_p = ctx.enter_context(tc.tile_pool(name="sm", bufs=2))
        ps = ctx.enter_context(tc.tile_pool(name="ps", bufs=1, space="PSUM"))

        ENG = {}

        # ---- loads: chunk-granular, all up front on the SP queue
        qraw = {}

        def load_chunk(b, t, h):
            qraw[(b, t, h)] = qraw_p.tile([128, 2048], f32, tag="qraw",
                                          name=f"qraw{b}_{t}_{h}")
            nc.sync.dma_start(
                qraw[(b, t, h)][:],
                x[b, 128 * t:128 * (t + 1), 2048 * h:2048 * (h + 1)],
            )

        load_chunk(*CHUNKS[0])
        load_chunk(*CHUNKS[1])
        ident = const_p.tile([128, 128], f32, tag="ident")
        nc.sync.dma_start(ident[:], ident_d)
        gam = const_p.tile([128, 1], f32, tag="gam")
        nc.sync.dma_start(gam[:], g128)
        for ch in CHUNKS[2:]:
            load_chunk(*ch)

        ident8 = const_p.tile([128, 128], f8, tag="ident8")
        nc.scalar.copy(ident8[:], ident[:])
        ones2 = const_p.tile([128, 2], f8, tag="ones2")
        nc.gpsimd.memset(ones2[:], 1.0)
        ones1 = const_p.tile([1, 128], f32, tag="ones1")
        nc.gpsimd.memset(ones1[:], 1.0)

        # ---- warm the PE clock while the first loads are in flight
        warm = ps.tile([128, 512], f32, tag="tp", bufs=2, name="warm")
        for w in range(8):
            nc.tensor.matmul(
                warm[:, 128 * (w % 4):128 * (w % 4 + 1)],
                ident[:],
                ident[:],
                is_transpose=True,
                skip_group_check=True,
            )

        x16 = [x16_p.tile([128, CT, N], f16, tag="x16", name=f"x16_{b}")
               for b in range(B)]
        q8 = [q8_p.tile([128, CT, N], f8, tag="q8", name=f"q8_{b}")
              for b in range(B)]
        qt = [None] * B
        U = [None] * B
        colrep = [None] * B
        E_tiles = {}
        Ecp = {}
        rmins = {}

        ENG["V"] = nc.vector
        ENG["A"] = nc.scalar
        ENG["P"] = nc.gpsimd

        def copy_op(e, dst, src):
            if e is nc.scalar:
                e.copy(dst, src)
            else:
                e.tensor_copy(dst, src)

        def cast_x16(b, t, h, eng=None):
            e = ENG[eng] if eng else (nc.vector if b == 0 else nc.gpsimd)
            copy_op(e, x16[b][:, t, 2048 * h:2048 * (h + 1)],
                    qraw[(b, t, h)][:])

        def front(b, t, h, drain_eng="A", x16_eng=None):
            """Per-chunk pipeline: q8 cast (DVE), x16 cast (DVE for b0,
            Pool for b1 unless overridden), 16 PE transposes, one drain."""
            src = qraw[(b, t, h)][:]
            copy_op(nc.vector, q8[b][:, t, 2048 * h:2048 * (h + 1)], src)
            if x16_eng != "skip":
                cast_x16(b, t, h, x16_eng)
            if qt[b] is None:
                qt[b] = qt_p.tile([128, NT, C], f8, tag="qt", name=f"qt{b}")
            # hw rule: fp8 transpose output element step must be 2, so a
            # 2KB psum bank holds 8 transposes in 2-byte lanes
            for g2 in range(2):
                tp = ps.tile([128, 2048], f8, tag="tp", bufs=2,
                             name=f"tp{b}_{t}_{h}_{g2}")
                for jj in range(8):
                    j = 16 * h + 8 * g2 + jj
                    out = (tp[:, 256 * jj:256 * (jj + 1)]
                           .rearrange("p (c two) -> p c two", two=2)[:, :, 0:1])
                    nc.tensor.matmul(
                        out,
                        q8[b][:, t, 128 * j:128 * (j + 1)],
                        ident8[:],
                        is_transpose=True,
                        skip_group_check=True,
                    )
                jlo = 16 * h + 8 * g2
                copy_op(ENG[drain_eng],
                        qt[b][:, jlo:jlo + 8, 128 * t:128 * (t + 1)]
                        .rearrange("p j (c o) -> p j c o", o=1),
                        tp[:].rearrange("p (j c two) -> p j c two",
                                        j=8, two=2)[:, :, :, 0:1])

        def energy_tile(b, t, j2lo, j2hi, part="L"):
            """Energy accumulation for c-columns region: L = [0:384] (needs
            only tiles 0-2 of qT for the moving operand), R = [384:512]
            (needs tile 3). Each region is its own PSUM accumulation group;
            R's group must start only after L's group stopped."""
            if (b, t) not in E_tiles:
                tag = "tp" if (b, t) in ((1, 2), (1, 3)) else "ebank"
                E_tiles[(b, t)] = ps.tile([128, C], f32, tag=tag,
                                          bufs=2, name=f"E{b}_{t}")
            Et = E_tiles[(b, t)]
            lo, hi = (0, 384) if part == "L" else (384, 512)
            for j2 in range(j2lo, j2hi):
                nc.tensor.matmul(
                    Et[:, lo:hi],
                    qt[b][:, 2 * j2:2 * j2 + 2, 128 * t:128 * (t + 1)],
                    qt[b][:, 2 * j2:2 * j2 + 2, lo:hi],
                    start=(j2 == 0),
                    stop=(j2 == NT // 2 - 1),
                    perf_mode=DR,
                    skip_group_check=True,
                )

        def stats_tile(b, t):
            """After E_t completes: row-min into the padded rmins tile and
            an SBUF copy of E_t (frees its PSUM bank)."""
            if b not in rmins:
                rmins[b] = sm_p.tile([128, 64], f32, tag="rmins",
                                     name=f"rmins{b}")
                nc.gpsimd.memset(rmins[b][:], 0.0)
                colrep[b] = rep_p.tile([128, C], f32, tag="colrep",
                                       name=f"colrep{b}")
            nc.vector.tensor_reduce(
                rmins[b][:, t:t + 1], E_tiles[(b, t)][:], axis=AX.X,
                op=OP.min,
            )
            if b not in Ecp:
                Ecp[b] = ecp_p.tile([128, CT, C], f32, tag="ecp",
                                    name=f"ecp{b}")
            nc.scalar.copy(Ecp[b][:, t, :], E_tiles[(b, t)][:])

        def vm_pro(b, m):
            """Lazy softmax for output tile m's 128 columns (broadcast the
            column mins, in-place subtract, exp to fp8), then the R
            reduction and the gamma/R scale. Emitted one m ahead of the
            chunk stream so this serial chain hides under it."""
            if U[b] is None:
                U[b] = u_p.tile([128, CT, C], f8, tag="u", name=f"U{b}")
            # 32x32 block-transpose of the min matrix shifted so column m
            # lands in block-column 0: row 32*blk of rmTm then holds the
            # column mins of channels 128m+32blk..+32 (rows 0/32/64/96 are
            # the only legal engine-AP start partitions)
            rmTm = sm_p.tile([128, 32], f32, tag="rmt", name=f"rmT{b}_{m}")
            nc.vector.transpose(rmTm[:], rmins[b][:, m:m + 32])
            for blk in range(4):
                nc.gpsimd.partition_broadcast(
                    colrep[b][:, 128 * m + 32 * blk:128 * m + 32 * (blk + 1)],
                    rmTm[32 * blk:32 * blk + 1, :],
                )
            crm = colrep[b][:, 128 * m:128 * (m + 1)]
            for t in range(CT):
                ect = Ecp[b][:, t, 128 * m:128 * (m + 1)]
                nc.vector.tensor_tensor(ect, crm, ect, op=OP.subtract)
            # clamp the exp argument to <= 0 so U <= 1 in fp8 even if the
            # hardware's min/broadcast path ever disagrees with the sim --
            # exp overflow would otherwise turn gamma=0 outputs into NaN
            nc.vector.tensor_scalar_min(
                Ecp[b][:, :, 128 * m:128 * (m + 1)],
                Ecp[b][:, :, 128 * m:128 * (m + 1)], 0.0
            )
            nc.scalar.activation(
                U[b][:, :, 128 * m:128 * (m + 1)],
                Ecp[b][:, :, 128 * m:128 * (m + 1)], ACT.Exp
            )

        def vm_chunks(b, m, plan):
            """R reduction, gamma/R scale, and the att@q chunk stream with
            fused scale + residual STTs. plan: 8 chars over {D,P,H} per
            512-wide chunk: direct STT on DVE / on Pool / hybrid ACT-scale
            + fp16 2x add."""
            Rp_host = ps.tile([128, 512], f32, tag="obank", bufs=4,
                              name=f"Rph{b}_{m}")
            Rp = Rp_host[:, 0:1]
            for k2 in range(CT // 2):
                nc.tensor.matmul(
                    Rp,
                    U[b][:, 2 * k2:2 * k2 + 2, 128 * m:128 * (m + 1)],
                    ones2[:].rearrange("p (a o) -> p a o", o=1),
                    start=(k2 == 0),
                    stop=(k2 == CT // 2 - 1),
                    perf_mode=DR,
                    skip_group_check=True,
                )
            # clamp R before the reciprocal: CoreSim guarantees R >= 1
            # (the argmin column contributes exp(0) = 1) but hardware fp8
            # numerics may differ, and 0 * inf would poison the residual
            Rsb = sm_p.tile([128, 1], f32, tag="rsb", name=f"rsb{b}_{m}")
            nc.vector.tensor_scalar_max(Rsb[:], Rp, 1e-38)
            rec = sm_p.tile([128, 1], f32, tag="rec", name=f"rec{b}_{m}")
            nc.vector.reciprocal(rec[:], Rsb[:])
            sc = sm_p.tile([128, 1], f32, tag="sc", name=f"sc{b}_{m}")
            nc.vector.tensor_scalar_mul(sc[:], rec[:], gam[:, 0:1])
            for half in range(2):  # two 2048-wide output halves per m
                osb = osb_p.tile([128, 2048], f16, tag="osb",
                                 name=f"osb{b}_{m}_{half}")
                for cc4 in range(4):  # four single-bank chunks per half
                    ch = 4 * half + cc4
                    kind = plan[ch]
                    # batch 1's stream may also rotate through the retired
                    # transpose/energy bank slots for extra pipeline depth
                    tag = "obank" if b == 0 else ("obank", "obank", "ebank",
                                                  "tp")[ch % 4]
                    On = ps.tile([128, 512], f32, tag=tag,
                                 bufs={"obank": 4, "ebank": 2, "tp": 2}[tag],
                                 name=f"O{b}_{m}_{ch}")
                    for k2 in range(CT // 2):
                        nc.tensor.matmul(
                            On[:],
                            U[b][:, 2 * k2:2 * k2 + 2,
                                 128 * m:128 * (m + 1)],
                            q8[b][:, 2 * k2:2 * k2 + 2,
                                  512 * ch:512 * (ch + 1)],
                            start=(k2 == 0),
                            stop=(k2 == CT // 2 - 1),
                            perf_mode=DR,
                            skip_group_check=True,
                        )
                    xs = x16[b][:, m, 512 * ch:512 * (ch + 1)]
                    dst = osb[:, 512 * cc4:512 * (cc4 + 1)]
                    if kind in ("H", "G"):
                        # gpsimd cannot read PSUM, so Pool joins the stream
                        # via the SBUF-side fp16 add after an ACT scale-copy
                        os16 = os16_p.tile([128, 512], f16, tag="os16",
                                           bufs=3, name=f"os16_{b}_{m}_{ch}")
                        nc.scalar.activation(
                            os16[:], On[:], ACT.Copy, scale=sc[:]
                        )
                        eng = nc.vector if kind == "H" else nc.gpsimd
                        eng.tensor_tensor(
                            dst, os16[:], xs, op=OP.add
                        )
                    else:
                        nc.vector.scalar_tensor_tensor(
                            dst, On[:], sc[:], xs,
                            op0=OP.mult, op1=OP.add,
                        )
                nc.sync.dma_start(
                    y[b, 128 * m:128 * (m + 1),
                      2048 * half:2048 * (half + 1)],
                    osb[:],
                )

        # ================= emission schedule =================
        front(0, 0, 0)
        front(0, 1, 0)
        front(1, 0, 0)
        front(0, 2, 0)
        front(0, 3, 0)
        energy_tile(0, 0, 0, 8, "L")
        front(1, 1, 0)
        energy_tile(0, 1, 0, 8, "L")
        front(0, 0, 1)
        energy_tile(0, 2, 0, 8, "L")
        front(0, 1, 1)
        energy_tile(0, 3, 0, 8, "L")
        front(1, 2, 0)
        front(0, 2, 1)
        front(0, 3, 1)
        front(1, 3, 0)
        for t in range(CT):           # b0 L-tails (gate: b0 t0-2 h1 drains)
            energy_tile(0, t, 8, 16, "L")
        for t in range(CT):           # b0 R columns + stats
            energy_tile(0, t, 0, 16, "R")
            stats_tile(0, t)
        energy_tile(1, 0, 0, 8, "L")
        energy_tile(1, 1, 0, 8, "L")
        front(1, 0, 1, drain_eng="A", x16_eng="V")
        front(1, 1, 1, drain_eng="A", x16_eng="V")
        vm_pro(0, 0)
        vm_chunks(0, 0, "DGDHDGDH")
        front(1, 2, 1, drain_eng="A", x16_eng="skip")
        energy_tile(1, 0, 8, 16, "L")
        energy_tile(1, 1, 8, 16, "L")
        vm_pro(0, 1)
        vm_chunks(0, 1, "GDHDGDHG")
        energy_tile(1, 2, 0, 16, "L")
        front(1, 3, 1, drain_eng="V", x16_eng="skip")
        vm_pro(0, 2)
        vm_chunks(0, 2, "DHDGDHDG")
        energy_tile(1, 3, 0, 16, "L")
        for t in range(CT):           # b1 R columns + stats
            energy_tile(1, t, 0, 16, "R")
            stats_tile(1, t)
        vm_pro(0, 3)
        vm_chunks(0, 3, "HGDGHDGD")
        vm_pro(1, 0)
        vm_chunks(1, 0, "DGDHDGDH")
        cast_x16(1, 2, 1, "P")
        vm_pro(1, 1)
        vm_chunks(1, 1, "GDHDGDHG")
        cast_x16(1, 3, 1, "P")
        vm_pro(1, 2)
        vm_chunks(1, 2, "DHDGDHDG")
        vm_pro(1, 3)
        vm_chunks(1, 3, "HGDGHDGD")

    nc.compile()
    return nc


def get_program():
    if "nc" not in _cache:
        _cache["nc"] = _build_program()
    return _cache["nc"]


def kernel(x, gamma):
    from concourse.bass_utils import run_bass_kernel_spmd

    nc = get_program()
    xr = np.ascontiguousarray(
        np.asarray(x, dtype=np.float32).reshape(B_TOTAL, C, N)
    )
    g = np.asarray(gamma, dtype=np.float32).reshape(1)
    g128 = np.ascontiguousarray(
        np.broadcast_to(g.reshape(1, 1), (128, 1))
    ).astype(np.float32)
    ident = np.eye(128, dtype=np.float32)
    in_maps = [
        {
            "x": xr[i * B:(i + 1) * B],
            "gamma128": g128,
            "ident": ident,
        }
        for i in range(NCORES)
    ]
    res = run_bass_kernel_spmd(nc, in_maps, list(range(NCORES))).results
    ys = [np.asarray(res[i]["y"], dtype=np.float32) for i in range(NCORES)]
    yf = np.concatenate(ys, axis=0)
    return yf.reshape(B_TOTAL, C, H, W).astype(np.float32)


# revision 38
# speedup vs baseline: 1.6025x; 1.0046x over previous
"""Trainium2 Bass kernel for the CAM (channel attention) module.

Computes, per batch element b:
    q = x[b].reshape(C, N)                      # C=512, N=4096
    E = q @ q.T                                 # C x C  (symmetric)
    att = softmax(rowmax(E) - E, axis=-1)       # == softmax(-E) row-wise
    out = gamma * (att @ q) + x[b]

Sharding: data-parallel over batch. 16 batch elements -> 2 per NeuronCore
across 8 cores. gamma replicated. No collectives.

Per-core roofline: the shared DMA engine pool moves x in (16.8 MB fp32)
and y out (8.4 MB fp16) at ~360 GB/s -> ~70 us. Everything else is
scheduled to hide under that:

  1. x streams in as [128,2048] fp32 chunks, interleaved 2:1 between the
     two batch elements so batch 0's pipeline finishes ~17us before the
     last load. Each chunk is cast to q8 (fp8e4, feeds all matmuls) and
     x16 (fp16 residual copy) from a small rotating fp32 buffer.
  2. qT[n_part, c] built with PE fp8 transposes (1 cycle/row); 8
     transposes pack one PSUM bank in 2-byte lanes (hw: fp8 transpose
     output element step must be 2), drained by one strided copy.
  3. Energy E tiles accumulate via fp8 DoubleRow matmuls (K=256 per
     instruction, 0.5 cycles/row) in two j-halves, the first starting
     before the last drains land. Each finished E tile is min-reduced
     and copied to SBUF immediately, freeing its PSUM bank for the next
     batch's transposes.
  4. Column-softmax without transposing attention: E is symmetric so row
     mins equal column mins; U[d,c] = exp(min_c - E[d,c]) <= 1 in fp8.
     The subtract runs in place on the SBUF copy of E.
  5. Value matmul att @ q via DoubleRow fp8 into single-bank PSUM
     chunks rotating 4+ deep (batch 1's chunks also reuse the retired
     transpose/energy bank slots); R_c = sum_d U[d,c] via a PE
     ones-reduction and one fused gamma/R divide (R >= 1 structurally).
     The scale + x residual drain each chunk directly
     (scalar_tensor_tensor on DVE/Pool) or as an ACT scale-copy plus a
     2x-mode fp16 DVE add, so all three elementwise engines share the
     output stream. The lazy softmax runs per output tile: a 32x32
     block-transpose of the min matrix, four gpsimd partition
     broadcasts, one fused fp16 subtract, one exp.
  6. All loads are issued on the SP queue before any store so waiting
     stores never starve the DMA pipe.

Output is written fp16 (max rel err ~5e-4 vs the fp32 reference, gate
is 2e-2); host casts back to fp32.
"""

import sys

import numpy as np

_REPO = "/opt/trn_rl_repo"
if _REPO not in sys.path:
    sys.path.insert(0, _REPO)

B_TOTAL, C, H, W = 16, 512, 64, 64
N = H * W          # 4096
NCORES = 8
B = B_TOTAL // NCORES  # batches per core = 2
CT = C // 128      # 4 c-tiles
NT = N // 128      # 32 n-tiles

# chunk arrival order: (b, t, h); h=0 halves first per batch so the
# first energy half can start early, 2:1 interleave favoring batch 0
CHUNKS = [
    (0, 0, 0), (0, 1, 0), (1, 0, 0), (0, 2, 0), (0, 3, 0), (1, 1, 0),
    (0, 0, 1), (0, 1, 1), (1, 2, 0), (0, 2, 1), (0, 3, 1), (1, 3, 0),
    (1, 0, 1), (1, 1, 1), (1, 2, 1), (1, 3, 1),
]

_cache = {}


def _build_program():
    import concourse.bass as bass
    import concourse.bacc as bacc
    import concourse.mybir as mybir
    import concourse.tile as tile
    from contextlib import ExitStack

    f32 = mybir.dt.float32
    f16 = mybir.dt.float16
    f8 = mybir.dt.float8e4
    AX = mybir.AxisListType
    OP = mybir.AluOpType
    ACT = mybir.ActivationFunctionType
    DR = mybir.MatmulPerfMode.DoubleRow

    nc = bacc.Bacc("TRN2", target_bir_lowering=False, debug=False)

    x = nc.dram_tensor("x", [B, C, N], f32, kind="ExternalInput").ap()
    g128 = nc.dram_tensor("gamma128", [128, 1], f32, kind="ExternalInput").ap()
    ident_d = nc.dram_tensor("ident", [128, 128], f32, kind="ExternalInput").ap()
    y = nc.dram_tensor("y", [B, C, N], f16, kind="ExternalOutput").ap()

    with ExitStack() as ctx:
        tc = ctx.enter_context(tile.TileContext(nc))
        const_p = ctx.enter_context(tc.tile_pool(name="const", bufs=1))
        qraw_p = ctx.enter_context(tc.tile_pool(name="qraw", bufs=3))
        x16_p = ctx.enter_context(tc.tile_pool(name="x16", bufs=2))
        q8_p = ctx.enter_context(tc.tile_pool(name="q8", bufs=2))
        qt_p = ctx.enter_context(tc.tile_pool(name="qt", bufs=2))
        u_p = ctx.enter_context(tc.tile_pool(name="u", bufs=2))
        rep_p = ctx.enter_context(tc.tile_pool(name="rep", bufs=1))
        ecp_p = ctx.enter_context(tc.tile_pool(name="ecp", bufs=2))
        os16_p = ctx.enter_context(tc.tile_pool(name="os16", bufs=2))
        osb_p = ctx.enter_context(tc.tile_pool(name="osb", bufs=4))
        sm_p = ctx.enter_context(tc.tile_pool(name="sm", bufs=2))
        ps = ctx.enter_context(tc.tile_pool(name="ps", bufs=1, space="PSUM"))

        ENG = {}

        # ---- loads: chunk-granular, all up front on the SP queue
        qraw = {}

        def load_chunk(b, t, h):
            qraw[(b, t, h)] = qraw_p.tile([128, 2048], f32, tag="qraw",
                                          name=f"qraw{b}_{t}_{h}")
            nc.sync.dma_start(
                qraw[(b, t, h)][:],
                x[b, 128 * t:128 * (t + 1), 2048 * h:2048 * (h + 1)],
            )

        load_chunk(*CHUNKS[0])
        load_chunk(*CHUNKS[1])
        ident = const_p.tile([128, 128], f32, tag="ident")
        nc.sync.dma_start(ident[:], ident_d)
        gam = const_p.tile([128, 1], f32, tag="gam")
        nc.sync.dma_start(gam[:], g128)
        for ch in CHUNKS[2:]:
            load_chunk(*ch)

        ident8 = const_p.tile([128, 128], f8, tag="ident8")
        nc.scalar.copy(ident8[:], ident[:])
        ones2 = const_p.tile([128, 2], f8, tag="ones2")
        nc.gpsimd.memset(ones2[:], 1.0)
        ones1 = const_p.tile([1, 128], f32, tag="ones1")
        nc.gpsimd.memset(ones1[:], 1.0)

        # ---- warm the PE clock while the first loads are in flight
        warm = ps.tile([128, 512], f32, tag="tp", bufs=2, name="warm")
        for w in range(8):
            nc.tensor.matmul(
                warm[:, 128 * (w % 4):128 * (w % 4 + 1)],
                ident[:],
                ident[:],
                is_transpose=True,
                skip_group_check=True,
            )

        x16 = [x16_p.tile([128, CT, N], f16, tag="x16", name=f"x16_{b}")
               for b in range(B)]
        q8 = [q8_p.tile([128, CT, N], f8, tag="q8", name=f"q8_{b}")
              for b in range(B)]
        qt = [None] * B
        U = [None] * B
        colrep = [None] * B
        E_tiles = {}
        Ecp = {}
        rmins = {}

        ENG["V"] = nc.vector
        ENG["A"] = nc.scalar
        ENG["P"] = nc.gpsimd

        def copy_op(e, dst, src):
            if e is nc.scalar:
                e.copy(dst, src)
            else:
                e.tensor_copy(dst, src)

        def cast_x16(b, t, h, eng=None):
            e = ENG[eng] if eng else (nc.vector if b == 0 else nc.gpsimd)
            copy_op(e, x16[b][:, t, 2048 * h:2048 * (h + 1)],
                    qraw[(b, t, h)][:])

        def front(b, t, h, drain_eng="A", x16_eng=None):
            """Per-chunk pipeline: q8 cast (DVE), x16 cast (DVE for b0,
            Pool for b1 unless overridden), 16 PE transposes, one drain."""
            src = qraw[(b, t, h)][:]
            copy_op(nc.vector, q8[b][:, t, 2048 * h:2048 * (h + 1)], src)
            if x16_eng != "skip":
                cast_x16(b, t, h, x16_eng)
            if qt[b] is None:
                qt[b] = qt_p.tile([128, NT, C], f8, tag="qt", name=f"qt{b}")
            # hw rule: fp8 transpose output element step must be 2, so a
            # 2KB psum bank holds 8 transposes in 2-byte lanes
            for g2 in range(2):
                tp = ps.tile([128, 2048], f8, tag="tp", bufs=2,
                             name=f"tp{b}_{t}_{h}_{g2}")
                for jj in range(8):
                    j = 16 * h + 8 * g2 + jj
                    out = (tp[:, 256 * jj:256 * (jj + 1)]
                           .rearrange("p (c two) -> p c two", two=2)[:, :, 0:1])
                    nc.tensor.matmul(
                        out,
                        q8[b][:, t, 128 * j:128 * (j + 1)],
                        ident8[:],
                        is_transpose=True,
                        skip_group_check=True,
                    )
                jlo = 16 * h + 8 * g2
                copy_op(ENG[drain_eng],
                        qt[b][:, jlo:jlo + 8, 128 * t:128 * (t + 1)]
                        .rearrange("p j (c o) -> p j c o", o=1),
                        tp[:].rearrange("p (j c two) -> p j c two",
                                        j=8, two=2)[:, :, :, 0:1])

        def energy_tile(b, t, j2lo, j2hi, part="L"):
            """Energy accumulation for c-columns region: L = [0:384] (needs
            only tiles 0-2 of qT for the moving operand), R = [384:512]
            (needs tile 3). Each region is its own PSUM accumulation group;
            R's group must start only after L's group stopped."""
            if (b, t) not in E_tiles:
                tag = "tp" if (b, t) in ((1, 2), (1, 3)) else "ebank"
                E_tiles[(b, t)] = ps.tile([128, C], f32, tag=tag,
                                          bufs=2, name=f"E{b}_{t}")
            Et = E_tiles[(b, t)]
            lo, hi = (0, 384) if part == "L" else (384, 512)
            for j2 in range(j2lo, j2hi):
                nc.tensor.matmul(
                    Et[:, lo:hi],
                    qt[b][:, 2 * j2:2 * j2 + 2, 128 * t:128 * (t + 1)],
                    qt[b][:, 2 * j2:2 * j2 + 2, lo:hi],
                    start=(j2 == 0),
                    stop=(j2 == NT // 2 - 1),
                    perf_mode=DR,
                    skip_group_check=True,
                )

        def stats_tile(b, t):
            """After E_t completes: row-min into the padded rmins tile and
            an SBUF copy of E_t (frees its PSUM bank)."""
            if b not in rmins:
                rmins[b] = sm_p.tile([128, 64], f32, tag="rmins",
                                     name=f"rmins{b}")
                nc.gpsimd.memset(rmins[b][:], 0.0)
                colrep[b] = rep_p.tile([128, C], f32, tag="colrep",
                                       name=f"colrep{b}")
            nc.vector.tensor_reduce(
                rmins[b][:, t:t + 1], E_tiles[(b, t)][:], axis=AX.X,
                op=OP.min,
            )
            if b not in Ecp:
                Ecp[b] = ecp_p.tile([128, CT, C], f32, tag="ecp",
                                    name=f"ecp{b}")
            nc.scalar.copy(Ecp[b][:, t, :], E_tiles[(b, t)][:])

        def vm_pro(b, m):
            """Lazy softmax for output tile m's 128 columns (broadcast the
            column mins, in-place subtract, exp to fp8), then the R
            reduction and the gamma/R scale. Emitted one m ahead of the
            chunk stream so this serial chain hides under it."""
            if U[b] is None:
                U[b] = u_p.tile([128, CT, C], f8, tag="u", name=f"U{b}")
            # 32x32 block-transpose of the min matrix shifted so column m
            # lands in block-column 0: row 32*blk of rmTm then holds the
            # column mins of channels 128m+32blk..+32 (rows 0/32/64/96 are
            # the only legal engine-AP start partitions)
            rmTm = sm_p.tile([128, 32], f32, tag="rmt", name=f"rmT{b}_{m}")
            nc.vector.transpose(rmTm[:], rmins[b][:, m:m + 32])
            for blk in range(4):
                nc.gpsimd.partition_broadcast(
                    colrep[b][:, 128 * m + 32 * blk:128 * m + 32 * (blk + 1)],
                    rmTm[32 * blk:32 * blk + 1, :],
                )
            crm = colrep[b][:, 128 * m:128 * (m + 1)]
            sub_eng = nc.vector
            for t in range(CT):
                ect = Ecp[b][:, t, 128 * m:128 * (m + 1)]
                sub_eng.tensor_tensor(ect, crm, ect, op=OP.subtract)
            # clamp the exp argument to <= 0 so U <= 1 in fp8 even if the
            # hardware's min/broadcast path ever disagrees with the sim --
            # exp overflow would otherwise turn gamma=0 outputs into NaN
            nc.vector.tensor_scalar_min(
                Ecp[b][:, :, 128 * m:128 * (m + 1)],
                Ecp[b][:, :, 128 * m:128 * (m + 1)], 0.0
            )
            nc.scalar.activation(
                U[b][:, :, 128 * m:128 * (m + 1)],
                Ecp[b][:, :, 128 * m:128 * (m + 1)], ACT.Exp
            )

        def vm_chunks(b, m, plan):
            """R reduction, gamma/R scale, and the att@q chunk stream with
            fused scale + residual STTs. plan: 8 chars over {D,P,H} per
            512-wide chunk: direct STT on DVE / on Pool / hybrid ACT-scale
            + fp16 2x add."""
            Rp_host = ps.tile([128, 512], f32, tag="obank", bufs=4,
                              name=f"Rph{b}_{m}")
            Rp = Rp_host[:, 0:1]
            for k2 in range(CT // 2):
                nc.tensor.matmul(
                    Rp,
                    U[b][:, 2 * k2:2 * k2 + 2, 128 * m:128 * (m + 1)],
                    ones2[:].rearrange("p (a o) -> p a o", o=1),
                    start=(k2 == 0),
                    stop=(k2 == CT // 2 - 1),
                    perf_mode=DR,
                    skip_group_check=True,
                )
            # clamp R before the reciprocal: CoreSim guarantees R >= 1
            # (the argmin column contributes exp(0) = 1) but hardware fp8
            # numerics may differ, and 0 * inf would poison the residual
            Rsb = sm_p.tile([128, 1], f32, tag="rsb", name=f"rsb{b}_{m}")
            nc.vector.tensor_scalar_max(Rsb[:], Rp, 1e-38)
            rec = sm_p.tile([128, 1], f32, tag="rec", name=f"rec{b}_{m}")
            nc.vector.reciprocal(rec[:], Rsb[:])
            sc = sm_p.tile([128, 1], f32, tag="sc", name=f"sc{b}_{m}")
            nc.vector.tensor_scalar_mul(sc[:], rec[:], gam[:, 0:1])
            for half in range(2):  # two 2048-wide output halves per m
                osb = osb_p.tile([128, 2048], f16, tag="osb",
                                 name=f"osb{b}_{m}_{half}")
                for cc4 in range(4):  # four single-bank chunks per half
                    ch = 4 * half + cc4
                    kind = plan[ch]
                    # batch 1's stream may also rotate through the retired
                    # transpose/energy bank slots for extra pipeline depth
                    tag = "obank" if b == 0 else ("obank", "obank", "ebank",
                                                  "tp")[ch % 4]
                    On = ps.tile([128, 512], f32, tag=tag,
                                 bufs={"obank": 4, "ebank": 2, "tp": 2}[tag],
                                 name=f"O{b}_{m}_{ch}")
                    for k2 in range(CT // 2):
                        nc.tensor.matmul(
                            On[:],
                            U[b][:, 2 * k2:2 * k2 + 2,
                                 128 * m:128 * (m + 1)],
                            q8[b][:, 2 * k2:2 * k2 + 2,
                                  512 * ch:512 * (ch + 1)],
                            start=(k2 == 0),
                            stop=(k2 == CT // 2 - 1),
                            perf_mode=DR,
                            skip_group_check=True,
                        )
                    xs = x16[b][:, m, 512 * ch:512 * (ch + 1)]
                    dst = osb[:, 512 * cc4:512 * (cc4 + 1)]
                    if kind in ("H", "G"):
                        # gpsimd cannot read PSUM, so Pool joins the stream
                        # via the SBUF-side fp16 add after an ACT scale-copy
                        os16 = os16_p.tile([128, 512], f16, tag="os16",
                                           bufs=6, name=f"os16_{b}_{m}_{ch}")
                        nc.scalar.activation(
                            os16[:], On[:], ACT.Copy, scale=sc[:]
                        )
                        eng = nc.vector if kind == "H" else nc.gpsimd
                        eng.tensor_tensor(
                            dst, os16[:], xs, op=OP.add
                        )
                    else:
                        nc.vector.scalar_tensor_tensor(
                            dst, On[:], sc[:], xs,
                            op0=OP.mult, op1=OP.add,
                        )
                nc.sync.dma_start(
                    y[b, 128 * m:128 * (m + 1),
                      2048 * half:2048 * (half + 1)],
                    osb[:],
                )

        # ================= emission schedule =================
        front(0, 0, 0)
        front(0, 1, 0)
        front(1, 0, 0)
        front(0, 2, 0)
        front(0, 3, 0)
        energy_tile(0, 0, 0, 8, "L")
        front(1, 1, 0)
        energy_tile(0, 1, 0, 8, "L")
        front(0, 0, 1)
        energy_tile(0, 2, 0, 8, "L")
        front(0, 1, 1)
        energy_tile(0, 3, 0, 8, "L")
        front(1, 2, 0)
        front(0, 2, 1)
        front(0, 3, 1)
        front(1, 3, 0)
        for t in range(CT):           # b0 L-tails (gate: b0 t0-2 h1 drains)
            energy_tile(0, t, 8, 16, "L")
        for t in range(CT):           # b0 R columns + stats
            energy_tile(0, t, 0, 16, "R")
            stats_tile(0, t)
        energy_tile(1, 0, 0, 8, "L")
        energy_tile(1, 1, 0, 8, "L")
        vm_pro(0, 0)
        front(1, 0, 1, drain_eng="A", x16_eng="V")
        front(1, 1, 1, drain_eng="A", x16_eng="V")
        vm_chunks(0, 0, "DGDHDGDH")
        front(1, 2, 1, drain_eng="A", x16_eng="skip")
        energy_tile(1, 0, 8, 16, "L")
        energy_tile(1, 1, 8, 16, "L")
        vm_pro(0, 1)
        vm_chunks(0, 1, "GDHDGDHG")
        energy_tile(1, 2, 0, 16, "L")
        front(1, 3, 1, drain_eng="V", x16_eng="skip")
        vm_pro(0, 2)
        vm_chunks(0, 2, "DHDGDHDG")
        energy_tile(1, 3, 0, 16, "L")
        for t in range(CT):           # b1 R columns + stats
            energy_tile(1, t, 0, 16, "R")
            stats_tile(1, t)
        vm_pro(0, 3)
        vm_chunks(0, 3, "HGDGHDGD")
        vm_pro(1, 0)
        vm_chunks(1, 0, "DGDHDGDH")
        cast_x16(1, 2, 1, "P")
        vm_pro(1, 1)
        vm_chunks(1, 1, "GDHDGDHG")
        cast_x16(1, 3, 1, "P")
        vm_pro(1, 2)
        vm_chunks(1, 2, "DHDGDHDG")
        vm_pro(1, 3)
        vm_chunks(1, 3, "HGDGHDGD")

    nc.compile()
    return nc


def get_program():
    if "nc" not in _cache:
        _cache["nc"] = _build_program()
    return _cache["nc"]


def kernel(x, gamma):
    from concourse.bass_utils import run_bass_kernel_spmd

    nc = get_program()
    xr = np.ascontiguousarray(
        np.asarray(x, dtype=np.float32).reshape(B_TOTAL, C, N)
    )
    g = np.asarray(gamma, dtype=np.float32).reshape(1)
    g128 = np.ascontiguousarray(
        np.broadcast_to(g.reshape(1, 1), (128, 1))
    ).astype(np.float32)
    ident = np.eye(128, dtype=np.float32)
    in_maps = [
        {
            "x": xr[i * B:(i + 1) * B],
            "gamma128": g128,
            "ident": ident,
        }
        for i in range(NCORES)
    ]
    res = run_bass_kernel_spmd(nc, in_maps, list(range(NCORES))).results
    ys = [np.asarray(res[i]["y"], dtype=np.float32) for i in range(NCORES)]
    yf = np.concatenate(ys, axis=0)
    return yf.reshape(B_TOTAL, C, H, W).astype(np.float32)


# revision 40
# speedup vs baseline: 1.6053x; 1.0017x over previous
"""Trainium2 Bass kernel for the CAM (channel attention) module.

Computes, per batch element b:
    q = x[b].reshape(C, N)                      # C=512, N=4096
    E = q @ q.T                                 # C x C  (symmetric)
    att = softmax(rowmax(E) - E, axis=-1)       # == softmax(-E) row-wise
    out = gamma * (att @ q) + x[b]

Sharding: data-parallel over batch. 16 batch elements -> 2 per NeuronCore
across 8 cores. gamma replicated. No collectives.

Per-core roofline: the shared DMA engine pool moves x in (16.8 MB fp32)
and y out (8.4 MB fp16) at ~360 GB/s -> ~70 us. Everything else is
scheduled to hide under that:

  1. x streams in as [128,2048] fp32 chunks, interleaved 2:1 between the
     two batch elements so batch 0's pipeline finishes ~17us before the
     last load. Each chunk is cast to q8 (fp8e4, feeds all matmuls) and
     x16 (fp16 residual copy) from a small rotating fp32 buffer.
  2. qT[n_part, c] built with PE fp8 transposes (1 cycle/row); 8
     transposes pack one PSUM bank in 2-byte lanes (hw: fp8 transpose
     output element step must be 2), drained by one strided copy.
  3. Energy E tiles accumulate via fp8 DoubleRow matmuls (K=256 per
     instruction, 0.5 cycles/row) in two j-halves, the first starting
     before the last drains land. Each finished E tile is min-reduced
     and copied to SBUF immediately, freeing its PSUM bank for the next
     batch's transposes.
  4. Column-softmax without transposing attention: E is symmetric so row
     mins equal column mins; U[d,c] = exp(min_c - E[d,c]) <= 1 in fp8.
     The subtract runs in place on the SBUF copy of E.
  5. Value matmul att @ q via DoubleRow fp8 into single-bank PSUM
     chunks rotating 4+ deep (batch 1's chunks also reuse the retired
     transpose/energy bank slots); R_c = sum_d U[d,c] via a PE
     ones-reduction and one fused gamma/R divide (R >= 1 structurally).
     The scale + x residual drain each chunk directly
     (scalar_tensor_tensor on DVE/Pool) or as an ACT scale-copy plus a
     2x-mode fp16 DVE add, so all three elementwise engines share the
     output stream. The lazy softmax runs per output tile: a 32x32
     block-transpose of the min matrix, four gpsimd partition
     broadcasts, one fused fp16 subtract, one exp.
  6. All loads are issued on the SP queue before any store so waiting
     stores never starve the DMA pipe.

Output is written fp16 (max rel err ~5e-4 vs the fp32 reference, gate
is 2e-2); host casts back to fp32.
"""

import sys

import numpy as np

_REPO = "/opt/trn_rl_repo"
if _REPO not in sys.path:
    sys.path.insert(0, _REPO)

B_TOTAL, C, H, W = 16, 512, 64, 64
N = H * W          # 4096
NCORES = 8
B = B_TOTAL // NCORES  # batches per core = 2
CT = C // 128      # 4 c-tiles
NT = N // 128      # 32 n-tiles

# chunk arrival order: (b, t, h); h=0 halves first per batch so the
# first energy half can start early, 2:1 interleave favoring batch 0
CHUNKS = [
    (0, 0, 0), (0, 1, 0), (1, 0, 0), (0, 2, 0), (0, 3, 0), (1, 1, 0),
    (0, 0, 1), (0, 1, 1), (1, 2, 0), (0, 2, 1), (0, 3, 1), (1, 3, 0),
    (1, 0, 1), (1, 1, 1), (1, 2, 1), (1, 3, 1),
]

_cache = {}


def _build_program():
    import concourse.bass as bass
    import concourse.bacc as bacc
    import concourse.mybir as mybir
    import concourse.tile as tile
    from contextlib import ExitStack

    f32 = mybir.dt.float32
    f16 = mybir.dt.float16
    f8 = mybir.dt.float8e4
    AX = mybir.AxisListType
    OP = mybir.AluOpType
    ACT = mybir.ActivationFunctionType
    DR = mybir.MatmulPerfMode.DoubleRow

    nc = bacc.Bacc("TRN2", target_bir_lowering=False, debug=False)

    x = nc.dram_tensor("x", [B, C, N], f32, kind="ExternalInput").ap()
    g128 = nc.dram_tensor("gamma128", [128, 1], f32, kind="ExternalInput").ap()
    ident_d = nc.dram_tensor("ident", [128, 128], f32, kind="ExternalInput").ap()
    y = nc.dram_tensor("y", [B, C, N], f16, kind="ExternalOutput").ap()

    with ExitStack() as ctx:
        tc = ctx.enter_context(tile.TileContext(nc))
        const_p = ctx.enter_context(tc.tile_pool(name="const", bufs=1))
        qraw_p = ctx.enter_context(tc.tile_pool(name="qraw", bufs=3))
        x16_p = ctx.enter_context(tc.tile_pool(name="x16", bufs=2))
        q8_p = ctx.enter_context(tc.tile_pool(name="q8", bufs=2))
        qt_p = ctx.enter_context(tc.tile_pool(name="qt", bufs=2))
        u_p = ctx.enter_context(tc.tile_pool(name="u", bufs=2))
        rep_p = ctx.enter_context(tc.tile_pool(name="rep", bufs=1))
        ecp_p = ctx.enter_context(tc.tile_pool(name="ecp", bufs=2))
        os16_p = ctx.enter_context(tc.tile_pool(name="os16", bufs=2))
        osb_p = ctx.enter_context(tc.tile_pool(name="osb", bufs=4))
        sm_p = ctx.enter_context(tc.tile_pool(name="sm", bufs=2))
        ps = ctx.enter_context(tc.tile_pool(name="ps", bufs=1, space="PSUM"))

        ENG = {}

        # ---- loads: chunk-granular, all up front on the SP queue
        qraw = {}

        def load_chunk(b, t, h):
            qraw[(b, t, h)] = qraw_p.tile([128, 2048], f32, tag="qraw",
                                          name=f"qraw{b}_{t}_{h}")
            nc.sync.dma_start(
                qraw[(b, t, h)][:],
                x[b, 128 * t:128 * (t + 1), 2048 * h:2048 * (h + 1)],
            )

        load_chunk(*CHUNKS[0])
        load_chunk(*CHUNKS[1])
        ident = const_p.tile([128, 128], f32, tag="ident")
        nc.sync.dma_start(ident[:], ident_d)
        gam = const_p.tile([128, 1], f32, tag="gam")
        nc.sync.dma_start(gam[:], g128)
        for ch in CHUNKS[2:]:
            load_chunk(*ch)

        ident8 = const_p.tile([128, 128], f8, tag="ident8")
        nc.scalar.copy(ident8[:], ident[:])
        ones2 = const_p.tile([128, 2], f8, tag="ones2")
        nc.gpsimd.memset(ones2[:], 1.0)
        ones1 = const_p.tile([1, 128], f32, tag="ones1")
        nc.gpsimd.memset(ones1[:], 1.0)

        # ---- warm the PE clock while the first loads are in flight
        warm = ps.tile([128, 512], f32, tag="tp", bufs=2, name="warm")
        for w in range(8):
            nc.tensor.matmul(
                warm[:, 128 * (w % 4):128 * (w % 4 + 1)],
                ident[:],
                ident[:],
                is_transpose=True,
                skip_group_check=True,
            )

        x16 = [x16_p.tile([128, CT, N], f16, tag="x16", name=f"x16_{b}")
               for b in range(B)]
        q8 = [q8_p.tile([128, CT, N], f8, tag="q8", name=f"q8_{b}")
              for b in range(B)]
        qt = [None] * B
        U = [None] * B
        colrep = [None] * B
        E_tiles = {}
        Ecp = {}
        rmins = {}

        ENG["V"] = nc.vector
        ENG["A"] = nc.scalar
        ENG["P"] = nc.gpsimd

        def copy_op(e, dst, src):
            if e is nc.scalar:
                e.copy(dst, src)
            else:
                e.tensor_copy(dst, src)

        def cast_x16(b, t, h, eng=None):
            e = ENG[eng] if eng else (nc.vector if b == 0 else nc.gpsimd)
            copy_op(e, x16[b][:, t, 2048 * h:2048 * (h + 1)],
                    qraw[(b, t, h)][:])

        def front(b, t, h, drain_eng="A", x16_eng=None, q8_eng="V"):
            """Per-chunk pipeline: q8 cast (DVE), x16 cast (DVE for b0,
            Pool for b1 unless overridden), 16 PE transposes, one drain."""
            src = qraw[(b, t, h)][:]
            copy_op(ENG[q8_eng], q8[b][:, t, 2048 * h:2048 * (h + 1)], src)
            if x16_eng != "skip":
                cast_x16(b, t, h, x16_eng)
            if qt[b] is None:
                qt[b] = qt_p.tile([128, NT, C], f8, tag="qt", name=f"qt{b}")
            # hw rule: fp8 transpose output element step must be 2, so a
            # 2KB psum bank holds 8 transposes in 2-byte lanes
            for g2 in range(2):
                tp = ps.tile([128, 2048], f8, tag="tp", bufs=2,
                             name=f"tp{b}_{t}_{h}_{g2}")
                for jj in range(8):
                    j = 16 * h + 8 * g2 + jj
                    out = (tp[:, 256 * jj:256 * (jj + 1)]
                           .rearrange("p (c two) -> p c two", two=2)[:, :, 0:1])
                    nc.tensor.matmul(
                        out,
                        q8[b][:, t, 128 * j:128 * (j + 1)],
                        ident8[:],
                        is_transpose=True,
                        skip_group_check=True,
                    )
                jlo = 16 * h + 8 * g2
                copy_op(ENG[drain_eng],
                        qt[b][:, jlo:jlo + 8, 128 * t:128 * (t + 1)]
                        .rearrange("p j (c o) -> p j c o", o=1),
                        tp[:].rearrange("p (j c two) -> p j c two",
                                        j=8, two=2)[:, :, :, 0:1])

        def energy_tile(b, t, j2lo, j2hi, part="L"):
            """Energy accumulation for c-columns region: L = [0:384] (needs
            only tiles 0-2 of qT for the moving operand), R = [384:512]
            (needs tile 3). Each region is its own PSUM accumulation group;
            R's group must start only after L's group stopped."""
            if (b, t) not in E_tiles:
                tag = "tp" if (b, t) in ((1, 2), (1, 3)) else "ebank"
                E_tiles[(b, t)] = ps.tile([128, C], f32, tag=tag,
                                          bufs=2, name=f"E{b}_{t}")
            Et = E_tiles[(b, t)]
            lo, hi = (0, 384) if part == "L" else (384, 512)
            for j2 in range(j2lo, j2hi):
                nc.tensor.matmul(
                    Et[:, lo:hi],
                    qt[b][:, 2 * j2:2 * j2 + 2, 128 * t:128 * (t + 1)],
                    qt[b][:, 2 * j2:2 * j2 + 2, lo:hi],
                    start=(j2 == 0),
                    stop=(j2 == NT // 2 - 1),
                    perf_mode=DR,
                    skip_group_check=True,
                )

        def stats_tile(b, t):
            """After E_t completes: row-min into the padded rmins tile and
            an SBUF copy of E_t (frees its PSUM bank)."""
            if b not in rmins:
                rmins[b] = sm_p.tile([128, 64], f32, tag="rmins",
                                     name=f"rmins{b}")
                nc.gpsimd.memset(rmins[b][:], 0.0)
                colrep[b] = rep_p.tile([128, C], f32, tag="colrep",
                                       name=f"colrep{b}")
            nc.vector.tensor_reduce(
                rmins[b][:, t:t + 1], E_tiles[(b, t)][:], axis=AX.X,
                op=OP.min,
            )
            if b not in Ecp:
                Ecp[b] = ecp_p.tile([128, CT, C], f32, tag="ecp",
                                    name=f"ecp{b}")
            nc.scalar.copy(Ecp[b][:, t, :], E_tiles[(b, t)][:])

        def vm_pro(b, m):
            """Lazy softmax for output tile m's 128 columns (broadcast the
            column mins, in-place subtract, exp to fp8), then the R
            reduction and the gamma/R scale. Emitted one m ahead of the
            chunk stream so this serial chain hides under it."""
            if U[b] is None:
                U[b] = u_p.tile([128, CT, C], f8, tag="u", name=f"U{b}")
            # 32x32 block-transpose of the min matrix shifted so column m
            # lands in block-column 0: row 32*blk of rmTm then holds the
            # column mins of channels 128m+32blk..+32 (rows 0/32/64/96 are
            # the only legal engine-AP start partitions)
            rmTm = sm_p.tile([128, 32], f32, tag="rmt", name=f"rmT{b}_{m}")
            nc.vector.transpose(rmTm[:], rmins[b][:, m:m + 32])
            for blk in range(4):
                nc.gpsimd.partition_broadcast(
                    colrep[b][:, 128 * m + 32 * blk:128 * m + 32 * (blk + 1)],
                    rmTm[32 * blk:32 * blk + 1, :],
                )
            crm = colrep[b][:, 128 * m:128 * (m + 1)]
            sub_eng = nc.vector
            for t in range(CT):
                ect = Ecp[b][:, t, 128 * m:128 * (m + 1)]
                sub_eng.tensor_tensor(ect, crm, ect, op=OP.subtract)
            # clamp the exp argument to <= 0 so U <= 1 in fp8 even if the
            # hardware's min/broadcast path ever disagrees with the sim --
            # exp overflow would otherwise turn gamma=0 outputs into NaN
            nc.vector.tensor_scalar_min(
                Ecp[b][:, :, 128 * m:128 * (m + 1)],
                Ecp[b][:, :, 128 * m:128 * (m + 1)], 0.0
            )
            nc.scalar.activation(
                U[b][:, :, 128 * m:128 * (m + 1)],
                Ecp[b][:, :, 128 * m:128 * (m + 1)], ACT.Exp
            )

        def vm_chunks(b, m, plan):
            """R reduction, gamma/R scale, and the att@q chunk stream with
            fused scale + residual STTs. plan: 8 chars over {D,P,H} per
            512-wide chunk: direct STT on DVE / on Pool / hybrid ACT-scale
            + fp16 2x add."""
            Rp_host = ps.tile([128, 512], f32, tag="obank", bufs=4,
                              name=f"Rph{b}_{m}")
            Rp = Rp_host[:, 0:1]
            for k2 in range(CT // 2):
                nc.tensor.matmul(
                    Rp,
                    U[b][:, 2 * k2:2 * k2 + 2, 128 * m:128 * (m + 1)],
                    ones2[:].rearrange("p (a o) -> p a o", o=1),
                    start=(k2 == 0),
                    stop=(k2 == CT // 2 - 1),
                    perf_mode=DR,
                    skip_group_check=True,
                )
            # clamp R before the reciprocal: CoreSim guarantees R >= 1
            # (the argmin column contributes exp(0) = 1) but hardware fp8
            # numerics may differ, and 0 * inf would poison the residual
            Rsb = sm_p.tile([128, 1], f32, tag="rsb", name=f"rsb{b}_{m}")
            nc.vector.tensor_scalar_max(Rsb[:], Rp, 1e-38)
            rec = sm_p.tile([128, 1], f32, tag="rec", name=f"rec{b}_{m}")
            nc.vector.reciprocal(rec[:], Rsb[:])
            sc = sm_p.tile([128, 1], f32, tag="sc", name=f"sc{b}_{m}")
            nc.vector.tensor_scalar_mul(sc[:], rec[:], gam[:, 0:1])
            for half in range(2):  # two 2048-wide output halves per m
                osb = osb_p.tile([128, 2048], f16, tag="osb",
                                 name=f"osb{b}_{m}_{half}")
                for cc4 in range(4):  # four single-bank chunks per half
                    ch = 4 * half + cc4
                    kind = plan[ch]
                    # batch 1's stream may also rotate through the retired
                    # transpose/energy bank slots for extra pipeline depth
                    tag = "obank" if b == 0 else ("obank", "obank", "ebank",
                                                  "tp")[ch % 4]
                    On = ps.tile([128, 512], f32, tag=tag,
                                 bufs={"obank": 4, "ebank": 2, "tp": 2}[tag],
                                 name=f"O{b}_{m}_{ch}")
                    for k2 in range(CT // 2):
                        nc.tensor.matmul(
                            On[:],
                            U[b][:, 2 * k2:2 * k2 + 2,
                                 128 * m:128 * (m + 1)],
                            q8[b][:, 2 * k2:2 * k2 + 2,
                                  512 * ch:512 * (ch + 1)],
                            start=(k2 == 0),
                            stop=(k2 == CT // 2 - 1),
                            perf_mode=DR,
                            skip_group_check=True,
                        )
                    xs = x16[b][:, m, 512 * ch:512 * (ch + 1)]
                    dst = osb[:, 512 * cc4:512 * (cc4 + 1)]
                    if kind in ("H", "G"):
                        # gpsimd cannot read PSUM, so Pool joins the stream
                        # via the SBUF-side fp16 add after an ACT scale-copy
                        os16 = os16_p.tile([128, 512], f16, tag="os16",
                                           bufs=6, name=f"os16_{b}_{m}_{ch}")
                        nc.scalar.activation(
                            os16[:], On[:], ACT.Copy, scale=sc[:]
                        )
                        eng = nc.vector if kind == "H" else nc.gpsimd
                        eng.tensor_tensor(
                            dst, os16[:], xs, op=OP.add
                        )
                    else:
                        nc.vector.scalar_tensor_tensor(
                            dst, On[:], sc[:], xs,
                            op0=OP.mult, op1=OP.add,
                        )
                nc.sync.dma_start(
                    y[b, 128 * m:128 * (m + 1),
                      2048 * half:2048 * (half + 1)],
                    osb[:],
                )

        # ================= emission schedule =================
        front(0, 0, 0)
        front(0, 1, 0)
        front(1, 0, 0)
        front(0, 2, 0)
        front(0, 3, 0)
        energy_tile(0, 0, 0, 8, "L")
        front(1, 1, 0)
        energy_tile(0, 1, 0, 8, "L")
        front(0, 0, 1)
        energy_tile(0, 2, 0, 8, "L")
        front(0, 1, 1)
        energy_tile(0, 3, 0, 8, "L")
        front(1, 2, 0)
        front(0, 2, 1)
        front(0, 3, 1)
        front(1, 3, 0)
        for t in range(CT):           # b0 L-tails (gate: b0 t0-2 h1 drains)
            energy_tile(0, t, 8, 16, "L")
        for t in range(CT):           # b0 R columns + stats
            energy_tile(0, t, 0, 16, "R")
            stats_tile(0, t)
        energy_tile(1, 0, 0, 8, "L")
        energy_tile(1, 1, 0, 8, "L")
        vm_pro(0, 0)
        front(1, 0, 1, drain_eng="A", x16_eng="P")
        front(1, 1, 1, drain_eng="A", x16_eng="P")
        vm_chunks(0, 0, "DGDHDGDH")
        front(1, 2, 1, drain_eng="A", x16_eng="skip")
        energy_tile(1, 0, 8, 16, "L")
        energy_tile(1, 1, 8, 16, "L")
        vm_pro(0, 1)
        vm_chunks(0, 1, "GDHDGDHG")
        energy_tile(1, 2, 0, 16, "L")
        front(1, 3, 1, drain_eng="V", x16_eng="skip")
        vm_pro(0, 2)
        vm_chunks(0, 2, "DHDGDHDG")
        energy_tile(1, 3, 0, 16, "L")
        for t in range(CT):           # b1 R columns + stats
            energy_tile(1, t, 0, 16, "R")
            stats_tile(1, t)
        vm_pro(0, 3)
        vm_chunks(0, 3, "HGDGHDGD")
        vm_pro(1, 0)
        vm_chunks(1, 0, "DGDHDGDH")
        cast_x16(1, 2, 1, "P")
        vm_pro(1, 1)
        vm_chunks(1, 1, "GDHDGDHG")
        cast_x16(1, 3, 1, "P")
        vm_pro(1, 2)
        vm_chunks(1, 2, "DHDGDHDG")
        vm_pro(1, 3)
        vm_chunks(1, 3, "HGDGHDGD")

    nc.compile()
    return nc


def get_program():
    if "nc" not in _cache:
        _cache["nc"] = _build_program()
    return _cache["nc"]


def kernel(x, gamma):
    from concourse.bass_utils import run_bass_kernel_spmd

    nc = get_program()
    xr = np.ascontiguousarray(
        np.asarray(x, dtype=np.float32).reshape(B_TOTAL, C, N)
    )
    g = np.asarray(gamma, dtype=np.float32).reshape(1)
    g128 = np.ascontiguousarray(
        np.broadcast_to(g.reshape(1, 1), (128, 1))
    ).astype(np.float32)
    ident = np.eye(128, dtype=np.float32)
    in_maps = [
        {
            "x": xr[i * B:(i + 1) * B],
            "gamma128": g128,
            "ident": ident,
        }
        for i in range(NCORES)
    ]
    res = run_bass_kernel_spmd(nc, in_maps, list(range(NCORES))).results
    ys = [np.asarray(res[i]["y"], dtype=np.float32) for i in range(NCORES)]
    yf = np.concatenate(ys, axis=0)
    return yf.reshape(B_TOTAL, C, H, W).astype(np.float32)


# revision 43
# speedup vs baseline: 1.6210x; 1.0098x over previous
"""Trainium2 Bass kernel for the CAM (channel attention) module.

Computes, per batch element b:
    q = x[b].reshape(C, N)                      # C=512, N=4096
    E = q @ q.T                                 # C x C  (symmetric)
    att = softmax(rowmax(E) - E, axis=-1)       # == softmax(-E) row-wise
    out = gamma * (att @ q) + x[b]

Sharding: data-parallel over batch. 16 batch elements -> 2 per NeuronCore
across 8 cores. gamma replicated. No collectives.

Per-core roofline: the shared DMA engine pool moves x in (16.8 MB fp32)
and y out (8.4 MB fp16) at ~360 GB/s -> ~70 us. Everything else is
scheduled to hide under that:

  1. x streams in as [128,2048] fp32 chunks, interleaved 2:1 between the
     two batch elements so batch 0's pipeline finishes ~17us before the
     last load. Each chunk is cast to q8 (fp8e4, feeds all matmuls) and
     x16 (fp16 residual copy) from a small rotating fp32 buffer.
  2. qT[n_part, c] built with PE fp8 transposes (1 cycle/row); 8
     transposes pack one PSUM bank in 2-byte lanes (hw: fp8 transpose
     output element step must be 2), drained by one strided copy.
  3. Energy E tiles accumulate via fp8 DoubleRow matmuls (K=256 per
     instruction, 0.5 cycles/row) in two j-halves, the first starting
     before the last drains land. Each finished E tile is min-reduced
     and copied to SBUF immediately, freeing its PSUM bank for the next
     batch's transposes.
  4. Column-softmax without transposing attention: E is symmetric so row
     mins equal column mins; U[d,c] = exp(min_c - E[d,c]) <= 1 in fp8.
     The subtract runs in place on the SBUF copy of E.
  5. Value matmul att @ q via DoubleRow fp8 into single-bank PSUM
     chunks rotating 4+ deep (batch 1's chunks also reuse the retired
     transpose/energy bank slots); R_c = sum_d U[d,c] via a PE
     ones-reduction and one fused gamma/R divide (R >= 1 structurally).
     The scale + x residual drain each chunk directly
     (scalar_tensor_tensor on DVE/Pool) or as an ACT scale-copy plus a
     2x-mode fp16 DVE add, so all three elementwise engines share the
     output stream. The lazy softmax runs per output tile: a 32x32
     block-transpose of the min matrix, four gpsimd partition
     broadcasts, one fused fp16 subtract, one exp.
  6. All loads are issued on the SP queue before any store so waiting
     stores never starve the DMA pipe.

Output is written fp16 (max rel err ~5e-4 vs the fp32 reference, gate
is 2e-2); host casts back to fp32.
"""

import sys

import numpy as np

_REPO = "/opt/trn_rl_repo"
if _REPO not in sys.path:
    sys.path.insert(0, _REPO)

B_TOTAL, C, H, W = 16, 512, 64, 64
N = H * W          # 4096
NCORES = 8
B = B_TOTAL // NCORES  # batches per core = 2
CT = C // 128      # 4 c-tiles
NT = N // 128      # 32 n-tiles

# chunk arrival order: (b, t, h); h=0 halves first per batch so the
# first energy half can start early, 2:1 interleave favoring batch 0
CHUNKS = [
    (0, 0, 0), (0, 1, 0), (1, 0, 0), (0, 2, 0), (0, 3, 0), (1, 1, 0),
    (0, 0, 1), (0, 1, 1), (1, 2, 0), (0, 2, 1), (0, 3, 1), (1, 3, 0),
    (1, 0, 1), (1, 1, 1), (1, 2, 1), (1, 3, 1),
]

_cache = {}


def _build_program():
    import concourse.bass as bass
    import concourse.bacc as bacc
    import concourse.mybir as mybir
    import concourse.tile as tile
    from contextlib import ExitStack

    f32 = mybir.dt.float32
    f16 = mybir.dt.float16
    f8 = mybir.dt.float8e4
    AX = mybir.AxisListType
    OP = mybir.AluOpType
    ACT = mybir.ActivationFunctionType
    DR = mybir.MatmulPerfMode.DoubleRow

    nc = bacc.Bacc("TRN2", target_bir_lowering=False, debug=False)

    x = nc.dram_tensor("x", [B, C, N], f32, kind="ExternalInput").ap()
    g128 = nc.dram_tensor("gamma128", [128, 1], f32, kind="ExternalInput").ap()
    ident_d = nc.dram_tensor("ident", [128, 128], f32, kind="ExternalInput").ap()
    y = nc.dram_tensor("y", [B, C, N], f16, kind="ExternalOutput").ap()

    with ExitStack() as ctx:
        tc = ctx.enter_context(tile.TileContext(nc))
        const_p = ctx.enter_context(tc.tile_pool(name="const", bufs=1))
        qraw_p = ctx.enter_context(tc.tile_pool(name="qraw", bufs=3))
        x16_p = ctx.enter_context(tc.tile_pool(name="x16", bufs=2))
        q8_p = ctx.enter_context(tc.tile_pool(name="q8", bufs=2))
        qt_p = ctx.enter_context(tc.tile_pool(name="qt", bufs=2))
        u_p = ctx.enter_context(tc.tile_pool(name="u", bufs=2))
        rep_p = ctx.enter_context(tc.tile_pool(name="rep", bufs=1))
        ecp_p = ctx.enter_context(tc.tile_pool(name="ecp", bufs=2))
        os16_p = ctx.enter_context(tc.tile_pool(name="os16", bufs=2))
        osb_p = ctx.enter_context(tc.tile_pool(name="osb", bufs=4))
        sm_p = ctx.enter_context(tc.tile_pool(name="sm", bufs=2))
        ps = ctx.enter_context(tc.tile_pool(name="ps", bufs=1, space="PSUM"))

        ENG = {}

        # ---- loads: chunk-granular, all up front on the SP queue
        qraw = {}

        def load_chunk(b, t, h):
            qraw[(b, t, h)] = qraw_p.tile([128, 2048], f32, tag="qraw",
                                          name=f"qraw{b}_{t}_{h}")
            nc.sync.dma_start(
                qraw[(b, t, h)][:],
                x[b, 128 * t:128 * (t + 1), 2048 * h:2048 * (h + 1)],
            )

        load_chunk(*CHUNKS[0])
        load_chunk(*CHUNKS[1])
        ident = const_p.tile([128, 128], f32, tag="ident")
        nc.sync.dma_start(ident[:], ident_d)
        gam = const_p.tile([128, 1], f32, tag="gam")
        nc.sync.dma_start(gam[:], g128)
        for ch in CHUNKS[2:]:
            load_chunk(*ch)

        ident8 = const_p.tile([128, 128], f8, tag="ident8")
        nc.scalar.copy(ident8[:], ident[:])
        ones2 = const_p.tile([128, 2], f8, tag="ones2")
        nc.gpsimd.memset(ones2[:], 1.0)
        ones1 = const_p.tile([1, 128], f32, tag="ones1")
        nc.gpsimd.memset(ones1[:], 1.0)

        # ---- warm the PE clock while the first loads are in flight
        warm = ps.tile([128, 512], f32, tag="tp", bufs=2, name="warm")
        for w in range(8):
            nc.tensor.matmul(
                warm[:, 128 * (w % 4):128 * (w % 4 + 1)],
                ident[:],
                ident[:],
                is_transpose=True,
                skip_group_check=True,
            )

        x16 = [x16_p.tile([128, CT, N], f16, tag="x16", name=f"x16_{b}")
               for b in range(B)]
        q8 = [q8_p.tile([128, CT, N], f8, tag="q8", name=f"q8_{b}")
              for b in range(B)]
        qt = [None] * B
        U = [None] * B
        colrep = [None] * B
        E_tiles = {}
        Ecp = {}
        rmins = {}

        ENG["V"] = nc.vector
        ENG["A"] = nc.scalar
        ENG["P"] = nc.gpsimd

        def copy_op(e, dst, src):
            if e is nc.scalar:
                e.copy(dst, src)
            else:
                e.tensor_copy(dst, src)

        def cast_x16(b, t, h, eng=None):
            e = ENG[eng] if eng else (nc.vector if b == 0 else nc.gpsimd)
            copy_op(e, x16[b][:, t, 2048 * h:2048 * (h + 1)],
                    qraw[(b, t, h)][:])

        def front(b, t, h, drain_eng="A", x16_eng=None, q8_eng="V"):
            """Per-chunk pipeline: q8 cast (DVE), x16 cast (DVE for b0,
            Pool for b1 unless overridden), 16 PE transposes, one drain."""
            src = qraw[(b, t, h)][:]
            copy_op(ENG[q8_eng], q8[b][:, t, 2048 * h:2048 * (h + 1)], src)
            if x16_eng != "skip":
                cast_x16(b, t, h, x16_eng)
            if qt[b] is None:
                qt[b] = qt_p.tile([128, NT, C], f8, tag="qt", name=f"qt{b}")
            # hw rule: fp8 transpose output element step must be 2, so a
            # 2KB psum bank holds 8 transposes in 2-byte lanes
            for g2 in range(2):
                tp = ps.tile([128, 2048], f8, tag="tp", bufs=2,
                             name=f"tp{b}_{t}_{h}_{g2}")
                for jj in range(8):
                    j = 16 * h + 8 * g2 + jj
                    out = (tp[:, 256 * jj:256 * (jj + 1)]
                           .rearrange("p (c two) -> p c two", two=2)[:, :, 0:1])
                    nc.tensor.matmul(
                        out,
                        q8[b][:, t, 128 * j:128 * (j + 1)],
                        ident8[:],
                        is_transpose=True,
                        skip_group_check=True,
                    )
                jlo = 16 * h + 8 * g2
                copy_op(ENG[drain_eng],
                        qt[b][:, jlo:jlo + 8, 128 * t:128 * (t + 1)]
                        .rearrange("p j (c o) -> p j c o", o=1),
                        tp[:].rearrange("p (j c two) -> p j c two",
                                        j=8, two=2)[:, :, :, 0:1])

        def energy_tile(b, t, j2lo, j2hi, part="L"):
            """Energy accumulation for c-columns region: L = [0:384] (needs
            only tiles 0-2 of qT for the moving operand), R = [384:512]
            (needs tile 3). Each region is its own PSUM accumulation group;
            R's group must start only after L's group stopped."""
            if (b, t) not in E_tiles:
                tag = "tp" if (b, t) in ((1, 2), (1, 3)) else "ebank"
                E_tiles[(b, t)] = ps.tile([128, C], f32, tag=tag,
                                          bufs=2, name=f"E{b}_{t}")
            Et = E_tiles[(b, t)]
            lo, hi = (0, 384) if part == "L" else (384, 512)
            for j2 in range(j2lo, j2hi):
                nc.tensor.matmul(
                    Et[:, lo:hi],
                    qt[b][:, 2 * j2:2 * j2 + 2, 128 * t:128 * (t + 1)],
                    qt[b][:, 2 * j2:2 * j2 + 2, lo:hi],
                    start=(j2 == 0),
                    stop=(j2 == NT // 2 - 1),
                    perf_mode=DR,
                    skip_group_check=True,
                )

        def stats_tile(b, t):
            """After E_t completes: row-min into the padded rmins tile and
            an SBUF copy of E_t (frees its PSUM bank)."""
            if b not in rmins:
                rmins[b] = sm_p.tile([128, 64], f32, tag="rmins",
                                     name=f"rmins{b}")
                nc.gpsimd.memset(rmins[b][:], 0.0)
                colrep[b] = rep_p.tile([128, C], f32, tag="colrep",
                                       name=f"colrep{b}")
            nc.vector.tensor_reduce(
                rmins[b][:, t:t + 1], E_tiles[(b, t)][:], axis=AX.X,
                op=OP.min,
            )
            if b not in Ecp:
                Ecp[b] = ecp_p.tile([128, CT, C], f32, tag="ecp",
                                    name=f"ecp{b}")
            nc.scalar.copy(Ecp[b][:, t, :], E_tiles[(b, t)][:])

        def vm_pro(b, m):
            """Lazy softmax for output tile m's 128 columns (broadcast the
            column mins, in-place subtract, exp to fp8), then the R
            reduction and the gamma/R scale. Emitted one m ahead of the
            chunk stream so this serial chain hides under it."""
            if U[b] is None:
                U[b] = u_p.tile([128, CT, C], f8, tag="u", name=f"U{b}")
            # 32x32 block-transpose of the min matrix shifted so column m
            # lands in block-column 0: row 32*blk of rmTm then holds the
            # column mins of channels 128m+32blk..+32 (rows 0/32/64/96 are
            # the only legal engine-AP start partitions)
            rmTm = sm_p.tile([128, 32], f32, tag="rmt", name=f"rmT{b}_{m}")
            nc.vector.transpose(rmTm[:], rmins[b][:, m:m + 32])
            for blk in range(4):
                nc.gpsimd.partition_broadcast(
                    colrep[b][:, 128 * m + 32 * blk:128 * m + 32 * (blk + 1)],
                    rmTm[32 * blk:32 * blk + 1, :],
                )
            crm = colrep[b][:, 128 * m:128 * (m + 1)]
            sub_eng = nc.vector
            for t in range(CT):
                ect = Ecp[b][:, t, 128 * m:128 * (m + 1)]
                sub_eng.tensor_tensor(ect, crm, ect, op=OP.subtract)
            # clamp the exp argument to <= 0 so U <= 1 in fp8 even if the
            # hardware's min/broadcast path ever disagrees with the sim --
            # exp overflow would otherwise turn gamma=0 outputs into NaN
            nc.vector.tensor_scalar_min(
                Ecp[b][:, :, 128 * m:128 * (m + 1)],
                Ecp[b][:, :, 128 * m:128 * (m + 1)], 0.0
            )
            nc.scalar.activation(
                U[b][:, :, 128 * m:128 * (m + 1)],
                Ecp[b][:, :, 128 * m:128 * (m + 1)], ACT.Exp
            )

        def vm_chunks(b, m, plan):
            """R reduction, gamma/R scale, and the att@q chunk stream with
            fused scale + residual STTs. plan: 8 chars over {D,P,H} per
            512-wide chunk: direct STT on DVE / on Pool / hybrid ACT-scale
            + fp16 2x add."""
            Rp_host = ps.tile([128, 512], f32, tag="obank", bufs=4,
                              name=f"Rph{b}_{m}")
            Rp = Rp_host[:, 0:1]
            for k2 in range(CT // 2):
                nc.tensor.matmul(
                    Rp,
                    U[b][:, 2 * k2:2 * k2 + 2, 128 * m:128 * (m + 1)],
                    ones2[:].rearrange("p (a o) -> p a o", o=1),
                    start=(k2 == 0),
                    stop=(k2 == CT // 2 - 1),
                    perf_mode=DR,
                    skip_group_check=True,
                )
            # clamp R before the reciprocal: CoreSim guarantees R >= 1
            # (the argmin column contributes exp(0) = 1) but hardware fp8
            # numerics may differ, and 0 * inf would poison the residual
            Rsb = sm_p.tile([128, 1], f32, tag="rsb", name=f"rsb{b}_{m}")
            nc.vector.tensor_scalar_max(Rsb[:], Rp, 1e-38)
            rec = sm_p.tile([128, 1], f32, tag="rec", name=f"rec{b}_{m}")
            nc.vector.reciprocal(rec[:], Rsb[:])
            sc = sm_p.tile([128, 1], f32, tag="sc", name=f"sc{b}_{m}")
            nc.vector.tensor_scalar_mul(sc[:], rec[:], gam[:, 0:1])
            for half in range(2):  # two 2048-wide output halves per m
                osb = osb_p.tile([128, 2048], f16, tag="osb",
                                 name=f"osb{b}_{m}_{half}")
                for cc4 in range(4):  # four single-bank chunks per half
                    ch = 4 * half + cc4
                    kind = plan[ch]
                    # batch 1's stream may also rotate through the retired
                    # transpose/energy bank slots for extra pipeline depth
                    tag = "obank" if b == 0 else ("obank", "obank", "ebank",
                                                  "tp")[ch % 4]
                    On = ps.tile([128, 512], f32, tag=tag,
                                 bufs={"obank": 4, "ebank": 2, "tp": 2}[tag],
                                 name=f"O{b}_{m}_{ch}")
                    for k2 in range(CT // 2):
                        nc.tensor.matmul(
                            On[:],
                            U[b][:, 2 * k2:2 * k2 + 2,
                                 128 * m:128 * (m + 1)],
                            q8[b][:, 2 * k2:2 * k2 + 2,
                                  512 * ch:512 * (ch + 1)],
                            start=(k2 == 0),
                            stop=(k2 == CT // 2 - 1),
                            perf_mode=DR,
                            skip_group_check=True,
                        )
                    xs = x16[b][:, m, 512 * ch:512 * (ch + 1)]
                    dst = osb[:, 512 * cc4:512 * (cc4 + 1)]
                    if kind in ("H", "G"):
                        # gpsimd cannot read PSUM, so Pool joins the stream
                        # via the SBUF-side fp16 add after an ACT scale-copy
                        os16 = os16_p.tile([128, 512], f16, tag="os16",
                                           bufs=6, name=f"os16_{b}_{m}_{ch}")
                        nc.scalar.activation(
                            os16[:], On[:], ACT.Copy, scale=sc[:]
                        )
                        eng = nc.vector if kind == "H" else nc.gpsimd
                        eng.tensor_tensor(
                            dst, os16[:], xs, op=OP.add
                        )
                    else:
                        nc.vector.scalar_tensor_tensor(
                            dst, On[:], sc[:], xs,
                            op0=OP.mult, op1=OP.add,
                        )
                nc.sync.dma_start(
                    y[b, 128 * m:128 * (m + 1),
                      2048 * half:2048 * (half + 1)],
                    osb[:],
                )

        # ================= emission schedule =================
        front(0, 0, 0)
        front(0, 1, 0)
        front(1, 0, 0)
        front(0, 2, 0)
        front(0, 3, 0)
        energy_tile(0, 0, 0, 8, "L")
        front(1, 1, 0)
        energy_tile(0, 1, 0, 8, "L")
        front(0, 0, 1)
        energy_tile(0, 2, 0, 8, "L")
        front(0, 1, 1)
        energy_tile(0, 3, 0, 8, "L")
        front(1, 2, 0)
        front(0, 2, 1)
        front(0, 3, 1)
        for t in range(CT):           # b0 L-tails (gate: b0 t0-2 h1 drains)
            energy_tile(0, t, 8, 16, "L")
        for t in range(CT):           # b0 R columns + stats
            energy_tile(0, t, 0, 16, "R")
            stats_tile(0, t)
        front(1, 3, 0)
        energy_tile(1, 0, 0, 8, "L")
        energy_tile(1, 1, 0, 8, "L")
        vm_pro(0, 0)
        front(1, 0, 1, drain_eng="A", x16_eng="P")
        front(1, 1, 1, drain_eng="A", x16_eng="P")
        vm_chunks(0, 0, "DGDHDGDH")
        front(1, 2, 1, drain_eng="A", x16_eng="skip")
        energy_tile(1, 0, 8, 16, "L")
        energy_tile(1, 1, 8, 16, "L")
        vm_pro(0, 1)
        vm_chunks(0, 1, "GDHDGDHG")
        energy_tile(1, 2, 0, 16, "L")
        front(1, 3, 1, drain_eng="V", x16_eng="skip")
        vm_pro(0, 2)
        vm_chunks(0, 2, "DHDGDHDG")
        energy_tile(1, 3, 0, 16, "L")
        for t in range(CT):           # b1 R columns + stats
            energy_tile(1, t, 0, 16, "R")
            stats_tile(1, t)
        vm_pro(0, 3)
        vm_chunks(0, 3, "HGDGHDGD")
        vm_pro(1, 0)
        vm_chunks(1, 0, "DGDHDGDH")
        cast_x16(1, 2, 1, "P")
        vm_pro(1, 1)
        vm_chunks(1, 1, "GDHDGDHG")
        cast_x16(1, 3, 1, "P")
        vm_pro(1, 2)
        vm_chunks(1, 2, "DHDGDHDG")
        vm_pro(1, 3)
        vm_chunks(1, 3, "HGDGHDGD")

    nc.compile()
    return nc


def get_program():
    if "nc" not in _cache:
        _cache["nc"] = _build_program()
    return _cache["nc"]


def kernel(x, gamma):
    from concourse.bass_utils import run_bass_kernel_spmd

    nc = get_program()
    xr = np.ascontiguousarray(
        np.asarray(x, dtype=np.float32).reshape(B_TOTAL, C, N)
    )
    g = np.asarray(gamma, dtype=np.float32).reshape(1)
    g128 = np.ascontiguousarray(
        np.broadcast_to(g.reshape(1, 1), (128, 1))
    ).astype(np.float32)
    ident = np.eye(128, dtype=np.float32)
    in_maps = [
        {
            "x": xr[i * B:(i + 1) * B],
            "gamma128": g128,
            "ident": ident,
        }
        for i in range(NCORES)
    ]
    res = run_bass_kernel_spmd(nc, in_maps, list(range(NCORES))).results
    ys = [np.asarray(res[i]["y"], dtype=np.float32) for i in range(NCORES)]
    yf = np.concatenate(ys, axis=0)
    return yf.reshape(B_TOTAL, C, H, W).astype(np.float32)


# revision 47
# speedup vs baseline: 1.6328x; 1.0072x over previous
"""Trainium2 Bass kernel for the CAM (channel attention) module.

Computes, per batch element b:
    q = x[b].reshape(C, N)                      # C=512, N=4096
    E = q @ q.T                                 # C x C  (symmetric)
    att = softmax(rowmax(E) - E, axis=-1)       # == softmax(-E) row-wise
    out = gamma * (att @ q) + x[b]

Sharding: data-parallel over batch. 16 batch elements -> 2 per NeuronCore
across 8 cores. gamma replicated. No collectives.

Per-core roofline: the shared DMA engine pool moves x in (16.8 MB fp32)
and y out (8.4 MB fp16) at ~360 GB/s -> ~70 us. Everything else is
scheduled to hide under that:

  1. x streams in as [128,2048] fp32 chunks, interleaved 2:1 between the
     two batch elements so batch 0's pipeline finishes ~17us before the
     last load. Each chunk is cast to q8 (fp8e4, feeds all matmuls) and
     x16 (fp16 residual copy) from a small rotating fp32 buffer.
  2. qT[n_part, c] built with PE fp8 transposes (1 cycle/row); 8
     transposes pack one PSUM bank in 2-byte lanes (hw: fp8 transpose
     output element step must be 2), drained by one strided copy.
  3. Energy E tiles accumulate via fp8 DoubleRow matmuls (K=256 per
     instruction, 0.5 cycles/row) in two j-halves, the first starting
     before the last drains land. Each finished E tile is min-reduced
     and copied to SBUF immediately, freeing its PSUM bank for the next
     batch's transposes.
  4. Column-softmax without transposing attention: E is symmetric so row
     mins equal column mins; U[d,c] = exp(min_c - E[d,c]) <= 1 in fp8.
     The subtract runs in place on the SBUF copy of E.
  5. Value matmul att @ q via DoubleRow fp8 into single-bank PSUM
     chunks rotating 4+ deep (batch 1's chunks also reuse the retired
     transpose/energy bank slots); R_c = sum_d U[d,c] via a PE
     ones-reduction and one fused gamma/R divide (R >= 1 structurally).
     The scale + x residual drain each chunk directly
     (scalar_tensor_tensor on DVE/Pool) or as an ACT scale-copy plus a
     2x-mode fp16 DVE add, so all three elementwise engines share the
     output stream. The lazy softmax runs per output tile: a 32x32
     block-transpose of the min matrix, four gpsimd partition
     broadcasts, one fused fp16 subtract, one exp.
  6. All loads are issued on the SP queue before any store so waiting
     stores never starve the DMA pipe.

Output is written fp16 (max rel err ~5e-4 vs the fp32 reference, gate
is 2e-2); host casts back to fp32.
"""

import sys

import numpy as np

_REPO = "/opt/trn_rl_repo"
if _REPO not in sys.path:
    sys.path.insert(0, _REPO)

B_TOTAL, C, H, W = 16, 512, 64, 64
N = H * W          # 4096
NCORES = 8
B = B_TOTAL // NCORES  # batches per core = 2
CT = C // 128      # 4 c-tiles
NT = N // 128      # 32 n-tiles

# chunk arrival order: (b, t, h); h=0 halves first per batch so the
# first energy half can start early, 2:1 interleave favoring batch 0
CHUNKS = [
    (0, 0, 0), (0, 1, 0), (1, 0, 0), (0, 2, 0), (0, 3, 0), (1, 1, 0),
    (0, 0, 1), (0, 1, 1), (1, 2, 0), (0, 2, 1), (0, 3, 1), (1, 3, 0),
    (1, 0, 1), (1, 1, 1), (1, 2, 1), (1, 3, 1),
]

_cache = {}


def _build_program():
    import concourse.bass as bass
    import concourse.bacc as bacc
    import concourse.mybir as mybir
    import concourse.tile as tile
    from contextlib import ExitStack

    f32 = mybir.dt.float32
    f16 = mybir.dt.float16
    f8 = mybir.dt.float8e4
    AX = mybir.AxisListType
    OP = mybir.AluOpType
    ACT = mybir.ActivationFunctionType
    DR = mybir.MatmulPerfMode.DoubleRow

    nc = bacc.Bacc("TRN2", target_bir_lowering=False, debug=False)

    x = nc.dram_tensor("x", [B, C, N], f32, kind="ExternalInput").ap()
    g128 = nc.dram_tensor("gamma128", [128, 1], f32, kind="ExternalInput").ap()
    ident_d = nc.dram_tensor("ident", [128, 128], f32, kind="ExternalInput").ap()
    y = nc.dram_tensor("y", [B, C, N], f16, kind="ExternalOutput").ap()

    with ExitStack() as ctx:
        tc = ctx.enter_context(tile.TileContext(nc))
        const_p = ctx.enter_context(tc.tile_pool(name="const", bufs=1))
        qraw_p = ctx.enter_context(tc.tile_pool(name="qraw", bufs=3))
        x16_p = ctx.enter_context(tc.tile_pool(name="x16", bufs=2))
        q8_p = ctx.enter_context(tc.tile_pool(name="q8", bufs=2))
        qt_p = ctx.enter_context(tc.tile_pool(name="qt", bufs=2))
        u_p = ctx.enter_context(tc.tile_pool(name="u", bufs=2))
        rep_p = ctx.enter_context(tc.tile_pool(name="rep", bufs=1))
        ecp_p = ctx.enter_context(tc.tile_pool(name="ecp", bufs=2))
        os16_p = ctx.enter_context(tc.tile_pool(name="os16", bufs=2))
        osb_p = ctx.enter_context(tc.tile_pool(name="osb", bufs=4))
        sm_p = ctx.enter_context(tc.tile_pool(name="sm", bufs=2))
        ps = ctx.enter_context(tc.tile_pool(name="ps", bufs=1, space="PSUM"))

        ENG = {}

        # ---- loads: chunk-granular, all up front on the SP queue
        qraw = {}

        def load_chunk(b, t, h):
            qraw[(b, t, h)] = qraw_p.tile([128, 2048], f32, tag="qraw",
                                          name=f"qraw{b}_{t}_{h}")
            nc.sync.dma_start(
                qraw[(b, t, h)][:],
                x[b, 128 * t:128 * (t + 1), 2048 * h:2048 * (h + 1)],
            )

        load_chunk(*CHUNKS[0])
        load_chunk(*CHUNKS[1])
        ident = const_p.tile([128, 128], f32, tag="ident")
        nc.sync.dma_start(ident[:], ident_d)
        gam = const_p.tile([128, 1], f32, tag="gam")
        nc.sync.dma_start(gam[:], g128)
        for ch in CHUNKS[2:]:
            load_chunk(*ch)

        ident8 = const_p.tile([128, 128], f8, tag="ident8")
        nc.scalar.copy(ident8[:], ident[:])
        ones2 = const_p.tile([128, 2], f8, tag="ones2")
        nc.gpsimd.memset(ones2[:], 1.0)
        ones1 = const_p.tile([1, 128], f32, tag="ones1")
        nc.gpsimd.memset(ones1[:], 1.0)

        # ---- warm the PE clock while the first loads are in flight
        warm = ps.tile([128, 512], f32, tag="tp", bufs=2, name="warm")
        for w in range(8):
            nc.tensor.matmul(
                warm[:, 128 * (w % 4):128 * (w % 4 + 1)],
                ident[:],
                ident[:],
                is_transpose=True,
                skip_group_check=True,
            )

        x16 = [x16_p.tile([128, CT, N], f16, tag="x16", name=f"x16_{b}")
               for b in range(B)]
        q8 = [q8_p.tile([128, CT, N], f8, tag="q8", name=f"q8_{b}")
              for b in range(B)]
        qt = [None] * B
        U = [None] * B
        colrep = [None] * B
        E_tiles = {}
        Ecp = {}
        rmins = {}

        ENG["V"] = nc.vector
        ENG["A"] = nc.scalar
        ENG["P"] = nc.gpsimd

        def copy_op(e, dst, src):
            if e is nc.scalar:
                e.copy(dst, src)
            else:
                e.tensor_copy(dst, src)

        def cast_x16(b, t, h, eng=None):
            e = ENG[eng] if eng else (nc.vector if b == 0 else nc.gpsimd)
            copy_op(e, x16[b][:, t, 2048 * h:2048 * (h + 1)],
                    qraw[(b, t, h)][:])

        def front(b, t, h, drain_eng="A", x16_eng=None, q8_eng="V"):
            """Per-chunk pipeline: q8 cast (DVE), x16 cast (DVE for b0,
            Pool for b1 unless overridden), 16 PE transposes, one drain."""
            src = qraw[(b, t, h)][:]
            copy_op(ENG[q8_eng], q8[b][:, t, 2048 * h:2048 * (h + 1)], src)
            if x16_eng != "skip":
                cast_x16(b, t, h, x16_eng)
            if qt[b] is None:
                qt[b] = qt_p.tile([128, NT, C], f8, tag="qt", name=f"qt{b}")
            # hw rule: fp8 transpose output element step must be 2, so a
            # 2KB psum bank holds 8 transposes in 2-byte lanes
            for g2 in range(2):
                tp = ps.tile([128, 2048], f8, tag="tp", bufs=2,
                             name=f"tp{b}_{t}_{h}_{g2}")
                for jj in range(8):
                    j = 16 * h + 8 * g2 + jj
                    out = (tp[:, 256 * jj:256 * (jj + 1)]
                           .rearrange("p (c two) -> p c two", two=2)[:, :, 0:1])
                    nc.tensor.matmul(
                        out,
                        q8[b][:, t, 128 * j:128 * (j + 1)],
                        ident8[:],
                        is_transpose=True,
                        skip_group_check=True,
                    )
                jlo = 16 * h + 8 * g2
                copy_op(ENG[drain_eng],
                        qt[b][:, jlo:jlo + 8, 128 * t:128 * (t + 1)]
                        .rearrange("p j (c o) -> p j c o", o=1),
                        tp[:].rearrange("p (j c two) -> p j c two",
                                        j=8, two=2)[:, :, :, 0:1])

        def energy_tile(b, t, j2lo, j2hi, part="L"):
            """Energy accumulation for c-columns region: L = [0:384] (needs
            only tiles 0-2 of qT for the moving operand), R = [384:512]
            (needs tile 3). Each region is its own PSUM accumulation group;
            R's group must start only after L's group stopped."""
            if (b, t) not in E_tiles:
                tag = "tp" if (b, t) in ((1, 2), (1, 3)) else "ebank"
                E_tiles[(b, t)] = ps.tile([128, C], f32, tag=tag,
                                          bufs=2, name=f"E{b}_{t}")
            Et = E_tiles[(b, t)]
            lo, hi = (0, 384) if part == "L" else (384, 512)
            for j2 in range(j2lo, j2hi):
                nc.tensor.matmul(
                    Et[:, lo:hi],
                    qt[b][:, 2 * j2:2 * j2 + 2, 128 * t:128 * (t + 1)],
                    qt[b][:, 2 * j2:2 * j2 + 2, lo:hi],
                    start=(j2 == 0),
                    stop=(j2 == NT // 2 - 1),
                    perf_mode=DR,
                    skip_group_check=True,
                )

        def stats_tile(b, t):
            """After E_t completes: row-min into the padded rmins tile and
            an SBUF copy of E_t (frees its PSUM bank)."""
            if b not in rmins:
                rmins[b] = sm_p.tile([128, 64], f32, tag="rmins",
                                     name=f"rmins{b}")
                nc.gpsimd.memset(rmins[b][:], 0.0)
                colrep[b] = rep_p.tile([128, C], f32, tag="colrep",
                                       name=f"colrep{b}")
            nc.vector.tensor_reduce(
                rmins[b][:, t:t + 1], E_tiles[(b, t)][:], axis=AX.X,
                op=OP.min,
            )
            if b not in Ecp:
                Ecp[b] = ecp_p.tile([128, CT, C], f32, tag="ecp",
                                    name=f"ecp{b}")
            nc.scalar.copy(Ecp[b][:, t, :], E_tiles[(b, t)][:])

        def vm_pro(b, m):
            """Lazy softmax for output tile m's 128 columns (broadcast the
            column mins, in-place subtract, exp to fp8), then the R
            reduction and the gamma/R scale. Emitted one m ahead of the
            chunk stream so this serial chain hides under it."""
            if U[b] is None:
                U[b] = u_p.tile([128, CT, C], f8, tag="u", name=f"U{b}")
            # 32x32 block-transpose of the min matrix shifted so column m
            # lands in block-column 0: row 32*blk of rmTm then holds the
            # column mins of channels 128m+32blk..+32 (rows 0/32/64/96 are
            # the only legal engine-AP start partitions)
            rmTm = sm_p.tile([128, 32], f32, tag="rmt", name=f"rmT{b}_{m}")
            nc.vector.transpose(rmTm[:], rmins[b][:, m:m + 32])
            for blk in range(4):
                nc.gpsimd.partition_broadcast(
                    colrep[b][:, 128 * m + 32 * blk:128 * m + 32 * (blk + 1)],
                    rmTm[32 * blk:32 * blk + 1, :],
                )
            crm = colrep[b][:, 128 * m:128 * (m + 1)]
            sub_eng = nc.vector
            for t in range(CT):
                ect = Ecp[b][:, t, 128 * m:128 * (m + 1)]
                sub_eng.tensor_tensor(ect, crm, ect, op=OP.subtract)
            # clamp the exp argument to <= 0 so U <= 1 in fp8 even if the
            # hardware's min/broadcast path ever disagrees with the sim --
            # exp overflow would otherwise turn gamma=0 outputs into NaN
            nc.vector.tensor_scalar_min(
                Ecp[b][:, :, 128 * m:128 * (m + 1)],
                Ecp[b][:, :, 128 * m:128 * (m + 1)], 0.0
            )
            nc.scalar.activation(
                U[b][:, :, 128 * m:128 * (m + 1)],
                Ecp[b][:, :, 128 * m:128 * (m + 1)], ACT.Exp
            )

        def vm_chunks(b, m, plan):
            """R reduction, gamma/R scale, and the att@q chunk stream with
            fused scale + residual STTs. plan: 8 chars over {D,P,H} per
            512-wide chunk: direct STT on DVE / on Pool / hybrid ACT-scale
            + fp16 2x add."""
            Rp_host = ps.tile([128, 512], f32, tag="obank", bufs=4,
                              name=f"Rph{b}_{m}")
            Rp = Rp_host[:, 0:1]
            for k2 in range(CT // 2):
                nc.tensor.matmul(
                    Rp,
                    U[b][:, 2 * k2:2 * k2 + 2, 128 * m:128 * (m + 1)],
                    ones2[:].rearrange("p (a o) -> p a o", o=1),
                    start=(k2 == 0),
                    stop=(k2 == CT // 2 - 1),
                    perf_mode=DR,
                    skip_group_check=True,
                )
            # clamp R before the reciprocal: CoreSim guarantees R >= 1
            # (the argmin column contributes exp(0) = 1) but hardware fp8
            # numerics may differ, and 0 * inf would poison the residual
            Rsb = sm_p.tile([128, 1], f32, tag="rsb", name=f"rsb{b}_{m}")
            nc.vector.tensor_scalar_max(Rsb[:], Rp, 1e-38)
            rec = sm_p.tile([128, 1], f32, tag="rec", name=f"rec{b}_{m}")
            nc.vector.reciprocal(rec[:], Rsb[:])
            sc = sm_p.tile([128, 1], f32, tag="sc", name=f"sc{b}_{m}")
            nc.vector.tensor_scalar_mul(sc[:], rec[:], gam[:, 0:1])
            for half in range(2):  # two 2048-wide output halves per m
                osb = osb_p.tile([128, 2048], f16, tag="osb",
                                 name=f"osb{b}_{m}_{half}")
                for cc4 in range(4):  # four single-bank chunks per half
                    ch = 4 * half + cc4
                    kind = plan[ch]
                    # batch 1's stream may also rotate through the retired
                    # transpose/energy bank slots for extra pipeline depth
                    tag = "obank" if b == 0 else ("obank", "obank", "ebank",
                                                  "tp")[ch % 4]
                    On = ps.tile([128, 512], f32, tag=tag,
                                 bufs={"obank": 4, "ebank": 2, "tp": 2}[tag],
                                 name=f"O{b}_{m}_{ch}")
                    for k2 in range(CT // 2):
                        nc.tensor.matmul(
                            On[:],
                            U[b][:, 2 * k2:2 * k2 + 2,
                                 128 * m:128 * (m + 1)],
                            q8[b][:, 2 * k2:2 * k2 + 2,
                                  512 * ch:512 * (ch + 1)],
                            start=(k2 == 0),
                            stop=(k2 == CT // 2 - 1),
                            perf_mode=DR,
                            skip_group_check=True,
                        )
                    xs = x16[b][:, m, 512 * ch:512 * (ch + 1)]
                    dst = osb[:, 512 * cc4:512 * (cc4 + 1)]
                    if kind in ("H", "G"):
                        # gpsimd cannot read PSUM, so Pool joins the stream
                        # via the SBUF-side fp16 add after an ACT scale-copy
                        os16 = os16_p.tile([128, 512], f16, tag="os16",
                                           bufs=6, name=f"os16_{b}_{m}_{ch}")
                        nc.scalar.activation(
                            os16[:], On[:], ACT.Copy, scale=sc[:]
                        )
                        eng = nc.vector if kind == "H" else nc.gpsimd
                        eng.tensor_tensor(
                            dst, os16[:], xs, op=OP.add
                        )
                    else:
                        nc.vector.scalar_tensor_tensor(
                            dst, On[:], sc[:], xs,
                            op0=OP.mult, op1=OP.add,
                        )
                    if cc4 % 2 == 1:  # ship each 1024-wide half as soon
                        q4 = cc4 // 2     # as its two chunks are drained
                        nc.sync.dma_start(
                            y[b, 128 * m:128 * (m + 1),
                              2048 * half + 1024 * q4:
                              2048 * half + 1024 * (q4 + 1)],
                            osb[:, 1024 * q4:1024 * (q4 + 1)],
                        )

        # ================= emission schedule =================
        front(0, 0, 0)
        front(0, 1, 0)
        front(1, 0, 0)
        front(0, 2, 0)
        front(0, 3, 0)
        energy_tile(0, 0, 0, 8, "L")
        front(1, 1, 0)
        energy_tile(0, 1, 0, 8, "L")
        front(0, 0, 1)
        energy_tile(0, 2, 0, 8, "L")
        front(0, 1, 1)
        energy_tile(0, 3, 0, 8, "L")
        front(1, 2, 0)
        front(0, 2, 1)
        front(0, 3, 1)
        for t in range(CT):           # b0 L-tails (gate: b0 t0-2 h1 drains)
            energy_tile(0, t, 8, 16, "L")
        for t in range(CT):           # b0 R columns + stats
            energy_tile(0, t, 0, 16, "R")
            stats_tile(0, t)
        front(1, 3, 0)
        energy_tile(1, 0, 0, 8, "L")
        energy_tile(1, 1, 0, 8, "L")
        vm_pro(0, 0)
        front(1, 0, 1, drain_eng="A", x16_eng="P")
        front(1, 1, 1, drain_eng="A", x16_eng="P")
        vm_chunks(0, 0, "DGDHDGDH")
        front(1, 2, 1, drain_eng="A", x16_eng="skip")
        energy_tile(1, 0, 8, 16, "L")
        energy_tile(1, 1, 8, 16, "L")
        vm_pro(0, 1)
        vm_chunks(0, 1, "GDHDGDHG")
        energy_tile(1, 2, 0, 16, "L")
        front(1, 3, 1, drain_eng="V", x16_eng="skip")
        vm_pro(0, 2)
        vm_chunks(0, 2, "DHDGDHDG")
        energy_tile(1, 3, 0, 16, "L")
        for t in range(CT):           # b1 R columns + stats
            energy_tile(1, t, 0, 16, "R")
            stats_tile(1, t)
        vm_pro(0, 3)
        vm_chunks(0, 3, "HGDGHDGD")
        vm_pro(1, 0)
        vm_chunks(1, 0, "DGDHDGDH")
        cast_x16(1, 2, 1, "P")
        vm_pro(1, 1)
        vm_chunks(1, 1, "GDHDGDHG")
        cast_x16(1, 3, 1, "P")
        vm_pro(1, 2)
        vm_chunks(1, 2, "DHDGDHDG")
        vm_pro(1, 3)
        vm_chunks(1, 3, "HGDGHDGD")

    nc.compile()
    return nc


def get_program():
    if "nc" not in _cache:
        _cache["nc"] = _build_program()
    return _cache["nc"]


def kernel(x, gamma):
    from concourse.bass_utils import run_bass_kernel_spmd

    nc = get_program()
    xr = np.ascontiguousarray(
        np.asarray(x, dtype=np.float32).reshape(B_TOTAL, C, N)
    )
    g = np.asarray(gamma, dtype=np.float32).reshape(1)
    g128 = np.ascontiguousarray(
        np.broadcast_to(g.reshape(1, 1), (128, 1))
    ).astype(np.float32)
    ident = np.eye(128, dtype=np.float32)
    in_maps = [
        {
            "x": xr[i * B:(i + 1) * B],
            "gamma128": g128,
            "ident": ident,
        }
        for i in range(NCORES)
    ]
    res = run_bass_kernel_spmd(nc, in_maps, list(range(NCORES))).results
    ys = [np.asarray(res[i]["y"], dtype=np.float32) for i in range(NCORES)]
    yf = np.concatenate(ys, axis=0)
    return yf.reshape(B_TOTAL, C, H, W).astype(np.float32)
